# revision 51
# baseline (speedup 1.0000x reference)
"""Trainium2 Bass kernel for a dense transformer block (B=2, T=2048, D=1024,
H=16, hd=64, MLP=4x), distributed across 8 NeuronCores.

Sharding: data-parallel over batch (cores 0-3 = batch 0, cores 4-7 = batch 1)
x tensor-parallel over heads (4 heads/core) for QKV+attention. The attention
output projection runs head-sharded per span (512 queries) and is re-sharded
token-parallel via one small bf16 ReduceScatter per span, pipelined behind the
following spans' attention compute, so the collective is (almost) never
exposed. Each core finishes LN2 + the MLP for the 4x128 token strips its
group's ReduceScatters deliver.

Performance structure (vs the naive version):
- All matmul operands are bf16; weights are cast+pre-transposed on the host so
  every weight DMA is contiguous per partition.
- LN scale/shift and biases are folded into the weight matrices host-side
  (wqkv <- g1*wqkv etc.), so on-chip LN is just (x-m)*rstd.
- Softmax denominators come from 64 replicated ones-columns appended to V:
  the PV matmul leaves ctx^T on partitions 0-63 and the denominator
  replicated on partitions 64-127, so normalization is one reciprocal + one
  multiply on the vector engine - no PE transposes in the softmax path.
- exp() runs on [128,1024] score-pair tiles to halve activation-engine
  instruction overhead; causal masking is a vector add of resident mask
  tiles (deduped across spans on the host).
- The tensor engine stream is kept dense (p-state ramp: 1.2GHz->2.4GHz after
  3us of continuous work): deep PSUM buffering, weights prefetched during
  attention, the last token strip's MLP work rides behind the final
  ReduceScatter while the other strips' MLP runs.
"""

from contextlib import ExitStack

import numpy as np

P = 128
B, T, D, HD = 2, 2048, 1024, 64
H = 16
MLPD = 4096
EPS = 1e-5
NCORES = 8
GROUP = 4          # cores per batch group
HLOC = H // GROUP  # heads per core
NT = T // P        # 16 token tiles
ND = D // P        # 8 feature tiles
NSP = T // 512     # 4 query spans
NFF = MLPD // P    # 32
SCALE = 1.0 / float(np.sqrt(HD))

_cache = {}


def _mask_structure(attn_mask):
    """Classify [span, kblock] and dedup the partial-block mask tiles.

    Returns (cls, uniq, umap):
      cls[(sp,kb)] in {"skip","free","partial"}
      uniq: list of np.ndarray [128,512] f32 additive masks (0 / -1e30)
      umap[(sp,kb)] = index into uniq for partial blocks
    """
    cls, uniq, umap, keys = {}, [], {}, {}
    m = np.asarray(attn_mask)
    for sp in range(NSP):
        qs = slice(sp * 512, sp * 512 + 512)
        for kb in range(NT):
            blk = m[qs, kb * P : kb * P + P]
            if blk.all():
                cls[(sp, kb)] = "skip"
            elif not blk.any():
                cls[(sp, kb)] = "free"
            else:
                cls[(sp, kb)] = "partial"
                add = np.where(blk.T, np.float32(0.0), np.float32(1.0))
                key = add.tobytes()
                if key not in keys:
                    keys[key] = len(uniq)
                    uniq.append(np.ascontiguousarray(add))
                umap[(sp, kb)] = keys[key]
    return cls, uniq, umap


def _build(mask_key):
    import concourse.bass as bass
    import concourse.mybir as mybir
    import concourse.tile as tile
    from concourse import bacc
    from concourse.masks import make_identity

    cls = dict(mask_key[0])
    umap = dict(mask_key[1])
    nuniq = mask_key[2]

    f32 = mybir.dt.float32
    bf16 = mybir.dt.bfloat16
    AF = mybir.ActivationFunctionType
    OP = mybir.AluOpType

    nc = bacc.Bacc("TRN2", target_bir_lowering=False, debug=False,
                   num_devices=NCORES)

    x_b = nc.dram_tensor("x_b", [T, D], f32, kind="ExternalInput")
    x_strips = nc.dram_tensor("x_strips", [4 * P, D], f32,
                              kind="ExternalInput")
    wqkv_p = nc.dram_tensor("wqkv_p", [P, ND * 768], bf16,
                            kind="ExternalInput")
    bqk_s = nc.dram_tensor("bqk_s", [512], f32, kind="ExternalInput")
    wout_p = nc.dram_tensor("wout_p", [P, 2 * D], bf16, kind="ExternalInput")
    mask_u = nc.dram_tensor("mask_u", [max(nuniq, 1) * P, 512], bf16,
                            kind="ExternalInput")
    w1_p = nc.dram_tensor("w1_p", [P, NFF * D], bf16, kind="ExternalInput")
    b1_eff = nc.dram_tensor("b1_eff", [MLPD], f32, kind="ExternalInput")
    w2_p = nc.dram_tensor("w2_p", [P, NFF * D], bf16, kind="ExternalInput")
    b2 = nc.dram_tensor("b2", [D], f32, kind="ExternalInput")
    out_strips = nc.dram_tensor("out_strips", [4 * P, D], f32,
                                kind="ExternalOutput")

    groups = [[0, 1, 2, 3], [4, 5, 6, 7]]

    def bcast_ap(handle, n):
        a = handle.ap()
        return bass.AP(tensor=a.tensor, offset=a.offset, ap=[[0, P], [1, n]])

    with tile.TileContext(nc) as tc, ExitStack() as st:
        consts = st.enter_context(tc.tile_pool(name="consts", bufs=1))
        identb = consts.tile([P, P], bf16)
        make_identity(nc, identb)
        eps_t = consts.tile([P, 1], f32)
        nc.vector.memset(eps_t[:], EPS)
        bqkt = consts.tile([P, 4], f32, name="bqkt")
        nc.gpsimd.dma_start(bqkt[:], bqk_s.ap().rearrange("(o p) -> p o", p=P))
        b1m = consts.tile([P, NFF], f32, name="b1m")
        nc.gpsimd.dma_start(b1m[:], b1_eff.ap().rearrange("(o p) -> p o", p=P))
        b2_bc = consts.tile([P, D], f32, name="b2bc")
        nc.gpsimd.dma_start(b2_bc[:], bcast_ap(b2, D))

        # Persistent state for stage D (residual + LN2 stats per strip).
        # x_mid is preloaded with x_strips (+bvout, host-folded); the
        # per-span ReduceScatter outputs land in rs_all and are added in.
        d_pool = st.enter_context(tc.tile_pool(name="dpool", bufs=1))
        x_mid = d_pool.tile([P, 4, D], f32)
        rs_all = d_pool.tile([P, 4, D], bf16)
        mvB = d_pool.tile([P, 4, 2], f32)

        # DRAM scratch for the per-span reduce-scatters
        dram_st = ExitStack()
        dram = dram_st.enter_context(tc.tile_pool(name="dram", bufs=1,
                                                  space="DRAM"))
        rs_in = [dram.tile([512, D], bf16, name=f"rsi{sp}")
                 for sp in range(NSP)]
        rs_out = [dram.tile([P, D], bf16, name=f"rso{sp}")
                  for sp in range(NSP)]

        # w1 + wout SBUF reserved up-front (pool stack is LIFO and they
        # outlive the attention-phase pools); their DMAs are emitted later.
        w_st = ExitStack()
        w1_pool = w_st.enter_context(tc.tile_pool(name="w1p", bufs=1))
        w1_sb = w1_pool.tile([P, NFF, ND, P], bf16)
        wo_st = ExitStack()
        wo_pool = wo_st.enter_context(tc.tile_pool(name="wop", bufs=1))
        wout_sb = wo_pool.tile([P, 2, D], bf16)

        mask_st = ExitStack()
        mask_pool = mask_st.enter_context(tc.tile_pool(name="maskp", bufs=1))
        mask_sb = mask_pool.tile([P, max(nuniq, 1), 512], bf16, name="msk")
        nc.gpsimd.dma_start(
            mask_sb[:], mask_u.ap().rearrange("(u p) q -> p u q", p=P))

        qv_st = ExitStack()
        qv_pool = qv_st.enter_context(tc.tile_pool(name="qv", bufs=1))
        qkT = qv_pool.tile([P, 4, T], bf16)
        v_sb = qv_pool.tile([P, NT, HLOC, 2 * HD], bf16)
        nc.gpsimd.memset(v_sb[:, :, :, HD : 2 * HD], 1.0)

        ctx_st = ExitStack()
        ctx_pool = ctx_st.enter_context(tc.tile_pool(name="ctxp", bufs=1))
        ctxT = ctx_pool.tile([P, 2, T], bf16)

        # ---------------- Stage A: LN1 + transpose -> hT ----------------
        ab_st = ExitStack()
        hT_pool = ab_st.enter_context(tc.tile_pool(name="hT", bufs=1))
        hT = hT_pool.tile([P, ND, T], bf16)
        wq_pool = ab_st.enter_context(tc.tile_pool(name="wqp", bufs=1))
        wqkv_sb = wq_pool.tile([P, ND, 768], bf16)
        nc.gpsimd.dma_start(wqkv_sb[:].rearrange("p a b -> p (a b)"),
                            wqkv_p[:, :])

        with tc.tile_pool(name="lnA", bufs=3) as lnA, \
             tc.tile_pool(name="lnAs", bufs=4) as lnAs, \
             tc.tile_pool(name="psA", bufs=3, space="PSUM") as psA, \
             tc.tile_pool(name="psB", bufs=4, space="PSUM") as psB:
            for nb in range(4):
                for tq in range(4):
                    tt = nb * 4 + tq
                    x_sb = lnA.tile([P, D], f32, name="x_sb")
                    nc.gpsimd.dma_start(x_sb[:], x_b[tt * P : tt * P + P, :])
                    stats = lnAs.tile([P, 2, 6], f32, name="stats")
                    xg = x_sb[:].rearrange("p (g d) -> p g d", g=2)
                    nc.vector.bn_stats(stats[:, 0, :], xg[:, 0, :])
                    nc.vector.bn_stats(stats[:, 1, :], xg[:, 1, :])
                    mv = lnAs.tile([P, 2], f32, name="mv")
                    nc.vector.bn_aggr(mv[:], stats[:])
                    rstd = lnAs.tile([P, 1], f32, name="rstd")
                    nc.scalar.activation(rstd[:], mv[:, 1:2], AF.Sqrt,
                                         bias=eps_t[:])
                    nc.vector.reciprocal(rstd[:], rstd[:])
                    negmr = lnAs.tile([P, 1], f32, name="negmr")
                    nc.vector.tensor_scalar(negmr[:], mv[:, 0:1], rstd[:],
                                            -1.0, op0=OP.mult, op1=OP.mult)
                    hno = lnA.tile([P, D], bf16, name="hno")
                    nc.scalar.activation(hno[:], x_sb[:], AF.Identity,
                                         bias=negmr[:], scale=rstd[:])
                    # all 8 transposes of this tile fill one PSUM bank,
                    # drained by a single wide copy (keeps the PE stream
                    # dense - no per-transpose slot round trips)
                    ptr8 = psA.tile([P, ND, P], bf16, name="ptr8")
                    for dd in range(ND):
                        nc.tensor.transpose(ptr8[:, dd, :],
                                            hno[:, dd * P : dd * P + P],
                                            identb[:])
                    nc.vector.tensor_copy(hT[:, :, tt * P : tt * P + P],
                                          ptr8[:])
                # -------- Stage B for this group of 4 token tiles --------
                for ft in range(4):
                    pq = psB.tile([P, 512], f32, name="pq")
                    for kk in range(ND):
                        nc.tensor.matmul(
                            pq[:], wqkv_sb[:, kk, ft * P : ft * P + P],
                            hT[:, kk, nb * 512 : nb * 512 + 512],
                            start=(kk == 0), stop=(kk == ND - 1))
                    nc.scalar.activation(
                        qkT[:, ft, nb * 512 : nb * 512 + 512], pq[:],
                        AF.Identity, bias=bqkt[:, ft : ft + 1])
                for tq in range(4):
                    tt = nb * 4 + tq
                    pv = psB.tile([P, 256], f32, name="pv", tag="pq")
                    for kk in range(ND):
                        nc.tensor.matmul(
                            pv[:], hT[:, kk, tt * P : tt * P + P],
                            wqkv_sb[:, kk, 512:768],
                            start=(kk == 0), stop=(kk == ND - 1))
                    nc.any.tensor_copy(
                        v_sb[:, tt, :, 0:HD],
                        pv[:].rearrange("p (h d) -> p h d", h=HLOC))

        ab_st.close()  # frees hT + wqkv (4MB + 1.5MB)

        # Weight DMAs for the next phases: wout now, w1 during attention.
        nc.gpsimd.dma_start(wout_sb[:].rearrange("p a b -> p (a b)"),
                            wout_p[:, :])
        for c in range(4):
            s = c * (NFF // 4)
            nc.gpsimd.dma_start(
                w1_sb[:, s : s + NFF // 4, :, :].rearrange(
                    "p a b c -> p (a b c)"),
                w1_p[:, s * D : (s + NFF // 4) * D])
        # Residual bases (x strips + folded bvout) preloaded into x_mid
        for sp in range(NSP):
            nc.gpsimd.dma_start(x_mid[:, sp, :],
                                x_strips[sp * P : sp * P + P, :])

        # ---------------- Stage C: attention ----------------
        attn_st = ExitStack()
        pTp = attn_st.enter_context(tc.tile_pool(name="pTp", bufs=10))
        rdp = attn_st.enter_context(tc.tile_pool(name="rdp", bufs=2))
        coutp = attn_st.enter_context(tc.tile_pool(name="coutp", bufs=6))
        psS = attn_st.enter_context(
            tc.tile_pool(name="psS", bufs=2, space="PSUM"))
        psC = attn_st.enter_context(
            tc.tile_pool(name="psC", bufs=2, space="PSUM"))
        psW = attn_st.enter_context(
            tc.tile_pool(name="psW", bufs=2, space="PSUM"))

        for sp in range(NSP):
            kbs = [kb for kb in range(NT) if cls[(sp, kb)] != "skip"]
            assert len(kbs) % 2 == 0
            npair = len(kbs) // 2
            for h in range(HLOC):
                po = HD * (h % 2)
                pctx = psC.tile([P, 512], f32, name="pctx")
                # scores burst: all pairs' QK^T + exp first (deep pT
                # buffering), then a dense PV accumulation burst - the PE
                # never idles pair-by-pair waiting on the activation engine
                pTs = []
                for i in range(npair):
                    kb0, kb1 = kbs[2 * i], kbs[2 * i + 1]
                    pp = psS.tile([P, 1024], f32, name="pp")
                    for half, kb in ((0, kb0), (1, kb1)):
                        nc.tensor.matmul(
                            pp[:, half * 512 : half * 512 + 512],
                            qkT[po : po + HD, 2 + h // 2, kb * P : kb * P + P],
                            qkT[po : po + HD, h // 2,
                                sp * 512 : sp * 512 + 512],
                            start=True, stop=True)
                    pT = pTp.tile([P, 1024], bf16, name="pT")
                    nc.scalar.activation(pT[:], pp[:], AF.Exp, scale=SCALE)
                    # causal masking: multiply by resident 0/1 bf16 tiles
                    # (zeroed entries drop out of both ctx and denominator)
                    u0 = umap.get((sp, kb0))
                    u1 = umap.get((sp, kb1))
                    if u0 is not None and u1 == u0 + 1:
                        nc.vector.tensor_tensor(
                            pT[:], pT[:],
                            mask_sb[:, u0 : u0 + 2, :].rearrange(
                                "p a b -> p (a b)"), op=OP.mult)
                    else:
                        for half, u in ((0, u0), (1, u1)):
                            if u is not None:
                                sl = slice(half * 512, half * 512 + 512)
                                nc.vector.tensor_tensor(
                                    pT[:, sl], pT[:, sl], mask_sb[:, u, :],
                                    op=OP.mult)
                    pTs.append((pT, kb0, kb1))
                for i, (pT, kb0, kb1) in enumerate(pTs):
                    for half, kb in ((0, kb0), (1, kb1)):
                        nc.tensor.matmul(
                            pctx[:], v_sb[:, kb, h, :],
                            pT[:, half * 512 : half * 512 + 512],
                            start=(i == 0 and half == 0),
                            stop=(i == npair - 1 and half == 1))
                den_sb = rdp.tile([HD, 512], f32, name="den_sb")
                nc.vector.tensor_copy(den_sb[:], pctx[HD : 2 * HD, :])
                rden = rdp.tile([HD, 512], f32, name="rden")
                nc.vector.reciprocal_approx_fast(rden[:], den_sb[:])
                nc.vector.tensor_tensor(
                    ctxT[po : po + HD, h // 2, sp * 512 : sp * 512 + 512],
                    pctx[0:HD, :], rden[:], op=OP.mult)
            # ---- output projection partials for this span + RS ----
            for tq in range(4):
                tt = sp * 4 + tq
                for fo in range(2):
                    pwo = psW.tile([P, 512], f32, name="pwo")
                    for kk in range(2):
                        nc.tensor.matmul(
                            pwo[:], ctxT[:, kk, tt * P : tt * P + P],
                            wout_sb[:, kk, fo * 512 : fo * 512 + 512],
                            start=(kk == 0), stop=(kk == 1))
                    c_out = coutp.tile([P, 512], bf16, name="c_out")
                    if fo == 0:
                        nc.vector.tensor_copy(c_out[:], pwo[:])
                    else:
                        nc.scalar.copy(c_out[:], pwo[:])
                    nc.sync.dma_start(
                        rs_in[sp][tq * P : tq * P + P,
                                  fo * 512 : fo * 512 + 512], c_out[:])
            nc.gpsimd.collective_compute(
                "ReduceScatter", mybir.AluOpType.add,
                ins=[rs_in[sp].opt()], outs=[rs_out[sp].opt()],
                replica_groups=groups)
            # readback (strip 3's is emitted after the w2 DMAs so the w2
            # load isn't queued behind the RS#3 wait on the gpsimd ring)
            if sp < 3:
                nc.gpsimd.dma_start(rs_all[:, sp, :], rs_out[sp][:, :])

        attn_st.close()
        ctx_st.close()
        qv_st.close()
        mask_st.close()
        wo_st.close()

        # w2 prefetch: starts once attention SBUF is released. Emitted on
        # the gpsimd ring BEFORE the strip-3 readback so it isn't queued
        # behind the RS#3 completion wait.
        w2_pool = w_st.enter_context(tc.tile_pool(name="w2p", bufs=1))
        w2_sb = w2_pool.tile([P, NFF, D], bf16)
        for c in range(8):
            s = c * (NFF // 8)
            nc.gpsimd.dma_start(
                w2_sb[:, s : s + NFF // 8, :].rearrange("p a b -> p (a b)"),
                w2_p[:, s * D : (s + NFF // 8) * D])
        nc.gpsimd.dma_start(rs_all[:, 3, :], rs_out[3][:, :])

        # ------------- Stage D: residual + LN2 -> h2T -------------
        mlp_st = ExitStack()
        mlp_pool = mlp_st.enter_context(tc.tile_pool(name="mlp", bufs=1))
        m1T = mlp_pool.tile([P, NFF, 512], bf16)
        h2T = mlp_pool.tile([P, ND, 512], bf16)

        lnB_st = ExitStack()
        lnB = lnB_st.enter_context(tc.tile_pool(name="lnB", bufs=2))
        lnBs = lnB_st.enter_context(tc.tile_pool(name="lnBs", bufs=4))

        def resid_stats(sp):
            nc.vector.tensor_add(x_mid[:, sp, :], x_mid[:, sp, :],
                                 rs_all[:, sp, :])
            statsB = lnBs.tile([P, 2, 6], f32, name="statsB")
            xmg = x_mid[:, sp, :].rearrange("p (g d) -> p g d", g=2)
            nc.vector.bn_stats(statsB[:, 0, :], xmg[:, 0, :])
            nc.vector.bn_stats(statsB[:, 1, :], xmg[:, 1, :])
            nc.vector.bn_aggr(mvB[:, sp, :], statsB[:])

        def ln2_strip(sp, trans_pool, trans_tag):
            rstdB = lnBs.tile([P, 1], f32, name="rstdB")
            nc.scalar.activation(rstdB[:], mvB[:, sp, 1:2], AF.Sqrt,
                                 bias=eps_t[:])
            nc.vector.reciprocal(rstdB[:], rstdB[:])
            h2 = lnB.tile([P, D], bf16, name="h2")
            nc.vector.tensor_scalar(h2[:], x_mid[:, sp, :], mvB[:, sp, 0:1],
                                    rstdB[:], op0=OP.subtract, op1=OP.mult)
            for dd in range(ND):
                ptr = trans_pool.tile([P, P], bf16, name=f"ptrD{trans_tag}",
                                      tag=trans_tag)
                nc.tensor.transpose(ptr[:], h2[:, dd * P : dd * P + P],
                                    identb[:])
                nc.any.tensor_copy(h2T[:, dd, sp * P : sp * P + P], ptr[:])

        with tc.tile_pool(name="psD2", bufs=2, space="PSUM") as psD2:
            for sp in range(3):
                resid_stats(sp)
                ln2_strip(sp, psD2, "ptrD")

        # ---------------- Stage E: MLP ----------------
        psE_st = ExitStack()
        psM1 = psE_st.enter_context(
            tc.tile_pool(name="psM1", bufs=2, space="PSUM"))
        psM2 = psE_st.enter_context(
            tc.tile_pool(name="psM2", bufs=1, space="PSUM"))

        # w1 pass1 (tokens 0:384) software-pipelined one ff ahead of the
        # w2 pass1 accumulation so the gelu handoff never stalls the PE
        pw2 = [[psM2.tile([P, 512], f32, name=f"pw2_{tl}_{fo}")
                for fo in range(2)] for tl in range(3)]

        def w1p1(ff):
            pm1 = psM1.tile([P, 384], f32, name="pm1", tag="pm1")
            for kk in range(ND):
                nc.tensor.matmul(pm1[:], w1_sb[:, ff, kk, :],
                                 h2T[:, kk, 0:384],
                                 start=(kk == 0), stop=(kk == ND - 1))
            nc.scalar.activation(m1T[:, ff, 0:384], pm1[:], AF.Gelu,
                                 bias=b1m[:, ff : ff + 1])

        def w2p1(ff):
            for tl in range(3):
                for fo in range(2):
                    nc.tensor.matmul(
                        pw2[tl][fo][:], m1T[:, ff, tl * P : tl * P + P],
                        w2_sb[:, ff, fo * 512 : fo * 512 + 512],
                        start=(ff == 0), stop=(ff == NFF - 1))

        w1p1(0)
        for ff in range(1, NFF):
            w1p1(ff)
            w2p1(ff - 1)
        w2p1(NFF - 1)
        # strip 3 residual+LN2 (waits on RS#3); emitted after the pass-1
        # loops so its Act/DVE work rides behind them in the queues
        resid_stats(3)
        ln2_strip(3, psM1, "pm1")

        with tc.tile_pool(name="outp", bufs=2) as outp:
            # finish + store strips 0-2
            for tl in range(3):
                for fo in range(2):
                    sl = slice(fo * 512, fo * 512 + 512)
                    o_sb = outp.tile([P, 512], f32, name="o_sb")
                    nc.vector.tensor_add(o_sb[:], pw2[tl][fo][:],
                                         x_mid[:, tl, sl])
                    nc.vector.tensor_add(o_sb[:], o_sb[:], b2_bc[:, sl])
                    nc.sync.dma_start(
                        out_strips[tl * P : tl * P + P, sl], o_sb[:])
            # strip 3: w1 pass2 pipelined one ff ahead of w2 pass2; w2
            # accumulators reuse the released pw2[0] slots
            pw2b = [psM2.tile([P, 512], f32, name=f"pw2b_{fo}",
                              tag=f"pw2_0_{fo}") for fo in range(2)]

            def w1p2(ff):
                pm1b = psM1.tile([P, P], f32, name="pm1b", tag="pm1")
                for kk in range(ND):
                    nc.tensor.matmul(pm1b[:], w1_sb[:, ff, kk, :],
                                     h2T[:, kk, 384:512],
                                     start=(kk == 0), stop=(kk == ND - 1))
                nc.scalar.activation(m1T[:, ff, 384:512], pm1b[:], AF.Gelu,
                                     bias=b1m[:, ff : ff + 1])

            def w2p2(ff):
                for fo in range(2):
                    nc.tensor.matmul(
                        pw2b[fo][:], m1T[:, ff, 384:512],
                        w2_sb[:, ff, fo * 512 : fo * 512 + 512],
                        start=(ff == 0), stop=(ff == NFF - 1))

            w1p2(0)
            for ff in range(1, NFF):
                w1p2(ff)
                w2p2(ff - 1)
            w2p2(NFF - 1)
            for fo in range(2):
                sl = slice(fo * 512, fo * 512 + 512)
                o_sb = outp.tile([P, 512], f32, name="o_sb")
                nc.vector.tensor_add(o_sb[:], pw2b[fo][:], x_mid[:, 3, sl])
                nc.vector.tensor_add(o_sb[:], o_sb[:], b2_bc[:, sl])
                nc.sync.dma_start(out_strips[3 * P : 4 * P, sl], o_sb[:])

        psE_st.close()
        lnB_st.close()
        mlp_st.close()
        w_st.close()
        dram_st.close()

    nc.compile()
    return nc


def _prepare_inputs(inputs):
    import ml_dtypes

    f64 = np.float64
    x = np.asarray(inputs["x"], dtype=np.float32)
    attn_mask = np.asarray(inputs["attn_mask"])
    ln1_g = np.asarray(inputs["ln1_g"], f64)
    ln1_b = np.asarray(inputs["ln1_b"], f64)
    ln2_g = np.asarray(inputs["ln2_g"], f64)
    ln2_b = np.asarray(inputs["ln2_b"], f64)
    wqkv = np.asarray(inputs["wqkv"], f64)
    bqkv = np.asarray(inputs["bqkv"], f64)
    wout = np.asarray(inputs["wout"], f64)
    bout = np.asarray(inputs["bout"], f64)
    w1 = np.asarray(inputs["w1"], f64)
    b1 = np.asarray(inputs["b1"], f64)
    w2 = np.asarray(inputs["w2"], f64)
    b2 = np.asarray(inputs["b2"], f64)

    bf = ml_dtypes.bfloat16

    wqkv_eff = ln1_g[:, None] * wqkv
    bqkv_eff = ln1_b @ wqkv + bqkv
    w1_eff = ln2_g[:, None] * w1
    b1_eff = ln2_b @ w1 + b1
    bvout = bqkv_eff[2 * D : 3 * D] @ wout + bout

    _, uniq, _ = _mask_structure(attn_mask)
    mask_u = (np.stack(uniq).reshape(-1, 512) if uniq
              else np.zeros((P, 512), np.float32))

    def pmajor3(w):  # [o*128, cols] -> [128, o*cols] partition-major
        o = w.shape[0] // P
        return np.ascontiguousarray(
            w.reshape(o, P, -1).transpose(1, 0, 2).reshape(P, -1))

    w1_prep = np.ascontiguousarray(
        w1_eff.astype(bf).reshape(ND, P, NFF, P).transpose(1, 2, 0, 3)
        .reshape(P, -1))
    w2_prep = np.ascontiguousarray(
        w2.astype(bf).reshape(NFF, P, D).transpose(1, 0, 2).reshape(P, -1))

    shared = {
        "mask_u": np.ascontiguousarray(mask_u.astype(bf)),
        "w1_p": w1_prep,
        "b1_eff": b1_eff.astype(np.float32),
        "w2_p": w2_prep,
        "b2": b2.astype(np.float32),
    }
    in_maps = []
    for c in range(NCORES):
        b, r = divmod(c, GROUP)
        hs = slice(r * HLOC * HD, (r + 1) * HLOC * HD)  # 256 features
        wq = wqkv_eff[:, 0:D][:, hs]
        wk = wqkv_eff[:, D : 2 * D][:, hs]
        wv = wqkv_eff[:, 2 * D : 3 * D][:, hs]
        wqkv_core = np.concatenate([wq, wk, wv], axis=1).astype(bf)
        wout_core = wout[hs, :].astype(bf)  # [256, D]
        strips = np.stack([
            x[b, sp * 512 + r * P : sp * 512 + (r + 1) * P, :]
            for sp in range(NSP)]).astype(f64) + bvout[None, None, :]
        in_maps.append(dict(
            shared,
            x_b=np.ascontiguousarray(x[b]),
            x_strips=np.ascontiguousarray(
                strips.reshape(4 * P, D).astype(np.float32)),
            wqkv_p=pmajor3(wqkv_core),
            bqk_s=np.ascontiguousarray(np.concatenate(
                [bqkv_eff[0:D][hs], bqkv_eff[D : 2 * D][hs]])
                .astype(np.float32)),
            wout_p=pmajor3(wout_core),
        ))
    return in_maps


def run(inputs, trace=False):
    from concourse.bass_utils import run_bass_kernel_spmd

    attn_mask = np.asarray(inputs["attn_mask"])
    cls, uniq, umap = _mask_structure(attn_mask)
    key = (tuple(sorted(cls.items())), tuple(sorted(umap.items())), len(uniq))
    if key not in _cache:
        _cache[key] = _build(key)
    nc = _cache[key]
    in_maps = _prepare_inputs(inputs)
    kw = {}
    if trace:
        kw = dict(trace=True, trace_cores=list(range(NCORES)))
    res = run_bass_kernel_spmd(nc, in_maps, core_ids=list(range(NCORES)), **kw)
    out = np.empty((B, T, D), np.float32)
    for c in range(NCORES):
        b, r = divmod(c, GROUP)
        strips = res.results[c]["out_strips"].reshape(NSP, P, D)
        for sp in range(NSP):
            out[b, sp * 512 + r * P : sp * 512 + (r + 1) * P, :] = strips[sp]
    return out, res


def kernel(**inputs):
    out, _ = run(inputs, trace=False)
    return out


# revision 52
# speedup vs baseline: 1.1158x; 1.1158x over previous
"""Trainium2 Bass kernel for a dense transformer block (B=2, T=2048, D=1024,
H=16, hd=64, MLP=4x), distributed across 8 NeuronCores.

Sharding: data-parallel over batch (cores 0-3 = batch 0, cores 4-7 = batch 1)
x tensor-parallel over heads (4 heads/core) for QKV+attention. The attention
output projection runs head-sharded per span (512 queries) and is re-sharded
token-parallel via one small bf16 ReduceScatter per span, pipelined behind the
following spans' attention compute, so the collective is (almost) never
exposed. Each core finishes LN2 + the MLP for the 4x128 token strips its
group's ReduceScatters deliver.

Performance structure (vs the naive version):
- All matmul operands are bf16; weights are cast+pre-transposed on the host so
  every weight DMA is contiguous per partition.
- LN scale/shift and biases are folded into the weight matrices host-side
  (wqkv <- g1*wqkv etc.), so on-chip LN is just (x-m)*rstd.
- Softmax denominators come from 64 replicated ones-columns appended to V:
  the PV matmul leaves ctx^T on partitions 0-63 and the denominator
  replicated on partitions 64-127, so normalization is one reciprocal + one
  multiply on the vector engine - no PE transposes in the softmax path.
- exp() runs on [128,1024] score-pair tiles to halve activation-engine
  instruction overhead; causal masking is a vector add of resident mask
  tiles (deduped across spans on the host).
- The tensor engine stream is kept dense (p-state ramp: 1.2GHz->2.4GHz after
  3us of continuous work): deep PSUM buffering, weights prefetched during
  attention, the last token strip's MLP work rides behind the final
  ReduceScatter while the other strips' MLP runs.
"""

from contextlib import ExitStack

import numpy as np

P = 128
B, T, D, HD = 2, 2048, 1024, 64
H = 16
MLPD = 4096
EPS = 1e-5
NCORES = 8
GROUP = 4          # cores per batch group
HLOC = H // GROUP  # heads per core
NT = T // P        # 16 token tiles
ND = D // P        # 8 feature tiles
NSP = T // 512     # 4 query spans
NFF = MLPD // P    # 32
SCALE = 1.0 / float(np.sqrt(HD))

_cache = {}


def _mask_structure(attn_mask):
    """Classify [span, kblock] and dedup the partial-block mask tiles.

    Returns (cls, uniq, umap):
      cls[(sp,kb)] in {"skip","free","partial"}
      uniq: list of np.ndarray [128,512] f32 additive masks (0 / -1e30)
      umap[(sp,kb)] = index into uniq for partial blocks
    """
    cls, uniq, umap, keys = {}, [], {}, {}
    m = np.asarray(attn_mask)
    for sp in range(NSP):
        qs = slice(sp * 512, sp * 512 + 512)
        for kb in range(NT):
            blk = m[qs, kb * P : kb * P + P]
            if blk.all():
                cls[(sp, kb)] = "skip"
            elif not blk.any():
                cls[(sp, kb)] = "free"
            else:
                cls[(sp, kb)] = "partial"
                add = np.where(blk.T, np.float32(0.0), np.float32(1.0))
                key = add.tobytes()
                if key not in keys:
                    keys[key] = len(uniq)
                    uniq.append(np.ascontiguousarray(add))
                umap[(sp, kb)] = keys[key]
    return cls, uniq, umap


def _build(mask_key):
    import concourse.bass as bass
    import concourse.mybir as mybir
    import concourse.tile as tile
    from concourse import bacc
    from concourse.masks import make_identity

    cls = dict(mask_key[0])
    umap = dict(mask_key[1])
    nuniq = mask_key[2]

    f32 = mybir.dt.float32
    bf16 = mybir.dt.bfloat16
    AF = mybir.ActivationFunctionType
    OP = mybir.AluOpType

    nc = bacc.Bacc("TRN2", target_bir_lowering=False, debug=False,
                   num_devices=NCORES)

    x_b = nc.dram_tensor("x_b", [T, D], f32, kind="ExternalInput")
    x_strips = nc.dram_tensor("x_strips", [4 * P, D], f32,
                              kind="ExternalInput")
    wqkv_p = nc.dram_tensor("wqkv_p", [P, ND * 768], bf16,
                            kind="ExternalInput")
    bqk_s = nc.dram_tensor("bqk_s", [512], f32, kind="ExternalInput")
    wout_p = nc.dram_tensor("wout_p", [P, 2 * D], bf16, kind="ExternalInput")
    mask_u = nc.dram_tensor("mask_u", [max(nuniq, 1) * P, 512], bf16,
                            kind="ExternalInput")
    w1_p = nc.dram_tensor("w1_p", [P, NFF * D], bf16, kind="ExternalInput")
    b1_eff = nc.dram_tensor("b1_eff", [MLPD], f32, kind="ExternalInput")
    w2_p = nc.dram_tensor("w2_p", [P, NFF * D], bf16, kind="ExternalInput")
    b2 = nc.dram_tensor("b2", [D], f32, kind="ExternalInput")
    out_strips = nc.dram_tensor("out_strips", [4 * P, D], f32,
                                kind="ExternalOutput")

    groups = [[0, 1, 2, 3], [4, 5, 6, 7]]

    def bcast_ap(handle, n):
        a = handle.ap()
        return bass.AP(tensor=a.tensor, offset=a.offset, ap=[[0, P], [1, n]])

    with tile.TileContext(nc) as tc, ExitStack() as st:
        consts = st.enter_context(tc.tile_pool(name="consts", bufs=1))
        identb = consts.tile([P, P], bf16)
        make_identity(nc, identb)
        eps_t = consts.tile([P, 1], f32)
        nc.vector.memset(eps_t[:], EPS)
        bqkt = consts.tile([P, 4], f32, name="bqkt")
        nc.gpsimd.dma_start(bqkt[:], bqk_s.ap().rearrange("(o p) -> p o", p=P))
        b1m = consts.tile([P, NFF], f32, name="b1m")
        nc.gpsimd.dma_start(b1m[:], b1_eff.ap().rearrange("(o p) -> p o", p=P))
        b2_bc = consts.tile([P, D], f32, name="b2bc")
        nc.gpsimd.dma_start(b2_bc[:], bcast_ap(b2, D))

        # Persistent state for stage D (residual + LN2 stats per strip).
        # x_mid is preloaded with x_strips (+bvout, host-folded); the
        # per-span ReduceScatter outputs land in rs_all and are added in.
        d_pool = st.enter_context(tc.tile_pool(name="dpool", bufs=1))
        x_mid = d_pool.tile([P, 4, D], f32)
        rs_all = d_pool.tile([P, 4, D], bf16)
        mvB = d_pool.tile([P, 4, 2], f32)

        # DRAM scratch for the per-span reduce-scatters
        dram_st = ExitStack()
        dram = dram_st.enter_context(tc.tile_pool(name="dram", bufs=1,
                                                  space="DRAM"))
        rs_in = [dram.tile([512, D], bf16, name=f"rsi{sp}")
                 for sp in range(NSP)]
        rs_out = [dram.tile([P, D], bf16, name=f"rso{sp}")
                  for sp in range(NSP)]

        # w1 + wout SBUF reserved up-front (pool stack is LIFO and they
        # outlive the attention-phase pools); their DMAs are emitted later.
        w_st = ExitStack()
        w1_pool = w_st.enter_context(tc.tile_pool(name="w1p", bufs=1))
        w1_sb = w1_pool.tile([P, NFF, ND, P], bf16)
        wo_st = ExitStack()
        wo_pool = wo_st.enter_context(tc.tile_pool(name="wop", bufs=1))
        wout_sb = wo_pool.tile([P, 2, D], bf16)

        mask_st = ExitStack()
        mask_pool = mask_st.enter_context(tc.tile_pool(name="maskp", bufs=1))
        mask_sb = mask_pool.tile([P, max(nuniq, 1), 512], bf16, name="msk")
        nc.gpsimd.dma_start(
            mask_sb[:], mask_u.ap().rearrange("(u p) q -> p u q", p=P))

        qv_st = ExitStack()
        qv_pool = qv_st.enter_context(tc.tile_pool(name="qv", bufs=1))
        qkT = qv_pool.tile([P, 4, T], bf16)
        v_sb = qv_pool.tile([P, NT, HLOC, 2 * HD], bf16)
        nc.gpsimd.memset(v_sb[:, :, :, HD : 2 * HD], 1.0)

        ctx_st = ExitStack()
        ctx_pool = ctx_st.enter_context(tc.tile_pool(name="ctxp", bufs=1))
        ctxT = ctx_pool.tile([P, 2, T], bf16)

        # ---------------- Stage A: LN1 + transpose -> hT ----------------
        ab_st = ExitStack()
        hT_pool = ab_st.enter_context(tc.tile_pool(name="hT", bufs=1))
        hT = hT_pool.tile([P, ND, T], bf16)
        wq_pool = ab_st.enter_context(tc.tile_pool(name="wqp", bufs=1))
        wqkv_sb = wq_pool.tile([P, ND, 768], bf16)
        nc.gpsimd.dma_start(wqkv_sb[:].rearrange("p a b -> p (a b)"),
                            wqkv_p[:, :])

        with tc.tile_pool(name="lnA", bufs=3) as lnA, \
             tc.tile_pool(name="lnAs", bufs=4) as lnAs, \
             tc.tile_pool(name="psA", bufs=3, space="PSUM") as psA, \
             tc.tile_pool(name="psB", bufs=4, space="PSUM") as psB:
            for nb in range(4):
                for tq in range(4):
                    tt = nb * 4 + tq
                    x_sb = lnA.tile([P, D], f32, name="x_sb")
                    nc.gpsimd.dma_start(x_sb[:], x_b[tt * P : tt * P + P, :])
                    stats = lnAs.tile([P, 2, 6], f32, name="stats")
                    xg = x_sb[:].rearrange("p (g d) -> p g d", g=2)
                    nc.vector.bn_stats(stats[:, 0, :], xg[:, 0, :])
                    nc.vector.bn_stats(stats[:, 1, :], xg[:, 1, :])
                    mv = lnAs.tile([P, 2], f32, name="mv")
                    nc.vector.bn_aggr(mv[:], stats[:])
                    rstd = lnAs.tile([P, 1], f32, name="rstd")
                    nc.scalar.activation(rstd[:], mv[:, 1:2], AF.Sqrt,
                                         bias=eps_t[:])
                    nc.vector.reciprocal(rstd[:], rstd[:])
                    negmr = lnAs.tile([P, 1], f32, name="negmr")
                    nc.vector.tensor_scalar(negmr[:], mv[:, 0:1], rstd[:],
                                            -1.0, op0=OP.mult, op1=OP.mult)
                    hno = lnA.tile([P, D], bf16, name="hno")
                    nc.scalar.activation(hno[:], x_sb[:], AF.Identity,
                                         bias=negmr[:], scale=rstd[:])
                    # all 8 transposes of this tile fill one PSUM bank,
                    # drained by a single wide copy (keeps the PE stream
                    # dense - no per-transpose slot round trips)
                    ptr8 = psA.tile([P, ND, P], bf16, name="ptr8")
                    for dd in range(ND):
                        nc.tensor.transpose(ptr8[:, dd, :],
                                            hno[:, dd * P : dd * P + P],
                                            identb[:])
                    nc.vector.tensor_copy(hT[:, :, tt * P : tt * P + P],
                                          ptr8[:])
                # -------- Stage B for this group of 4 token tiles --------
                for ft in range(4):
                    pq = psB.tile([P, 512], f32, name="pq")
                    for kk in range(ND):
                        nc.tensor.matmul(
                            pq[:], wqkv_sb[:, kk, ft * P : ft * P + P],
                            hT[:, kk, nb * 512 : nb * 512 + 512],
                            start=(kk == 0), stop=(kk == ND - 1))
                    nc.scalar.activation(
                        qkT[:, ft, nb * 512 : nb * 512 + 512], pq[:],
                        AF.Identity, bias=bqkt[:, ft : ft + 1])
                for tq in range(4):
                    tt = nb * 4 + tq
                    pv = psB.tile([P, 256], f32, name="pv", tag="pq")
                    for kk in range(ND):
                        nc.tensor.matmul(
                            pv[:], hT[:, kk, tt * P : tt * P + P],
                            wqkv_sb[:, kk, 512:768],
                            start=(kk == 0), stop=(kk == ND - 1))
                    nc.any.tensor_copy(
                        v_sb[:, tt, :, 0:HD],
                        pv[:].rearrange("p (h d) -> p h d", h=HLOC))

        ab_st.close()  # frees hT + wqkv (4MB + 1.5MB)

        # Weight DMAs for the next phases: wout now, w1 during attention.
        nc.gpsimd.dma_start(wout_sb[:].rearrange("p a b -> p (a b)"),
                            wout_p[:, :])
        for c in range(4):
            s = c * (NFF // 4)
            nc.gpsimd.dma_start(
                w1_sb[:, s : s + NFF // 4, :, :].rearrange(
                    "p a b c -> p (a b c)"),
                w1_p[:, s * D : (s + NFF // 4) * D])
        # Residual bases (x strips + folded bvout) preloaded into x_mid
        for sp in range(NSP):
            nc.gpsimd.dma_start(x_mid[:, sp, :],
                                x_strips[sp * P : sp * P + P, :])

        # ---------------- Stage C: attention ----------------
        attn_st = ExitStack()
        pTp = attn_st.enter_context(tc.tile_pool(name="pTp", bufs=10))
        rdp = attn_st.enter_context(tc.tile_pool(name="rdp", bufs=2))
        coutp = attn_st.enter_context(tc.tile_pool(name="coutp", bufs=6))
        psS = attn_st.enter_context(
            tc.tile_pool(name="psS", bufs=3, space="PSUM"))
        psC = attn_st.enter_context(
            tc.tile_pool(name="psC", bufs=2, space="PSUM"))

        for sp in range(NSP):
            kbs = [kb for kb in range(NT) if cls[(sp, kb)] != "skip"]
            assert len(kbs) % 2 == 0
            npair = len(kbs) // 2
            for h in range(HLOC):
                po = HD * (h % 2)
                pctx = psC.tile([P, 512], f32, name="pctx")
                # scores burst: all pairs' QK^T + exp first (deep pT
                # buffering), then a dense PV accumulation burst - the PE
                # never idles pair-by-pair waiting on the activation engine
                pTs = []
                for i in range(npair):
                    kb0, kb1 = kbs[2 * i], kbs[2 * i + 1]
                    pp = psS.tile([P, 1024], f32, name="pp")
                    for half, kb in ((0, kb0), (1, kb1)):
                        nc.tensor.matmul(
                            pp[:, half * 512 : half * 512 + 512],
                            qkT[po : po + HD, 2 + h // 2, kb * P : kb * P + P],
                            qkT[po : po + HD, h // 2,
                                sp * 512 : sp * 512 + 512],
                            start=True, stop=True)
                    pT = pTp.tile([P, 1024], bf16, name="pT")
                    nc.scalar.activation(pT[:], pp[:], AF.Exp, scale=SCALE)
                    # causal masking: multiply by resident 0/1 bf16 tiles
                    # (zeroed entries drop out of both ctx and denominator)
                    u0 = umap.get((sp, kb0))
                    u1 = umap.get((sp, kb1))
                    if u0 is not None and u1 == u0 + 1:
                        nc.vector.tensor_tensor(
                            pT[:], pT[:],
                            mask_sb[:, u0 : u0 + 2, :].rearrange(
                                "p a b -> p (a b)"), op=OP.mult)
                    else:
                        for half, u in ((0, u0), (1, u1)):
                            if u is not None:
                                sl = slice(half * 512, half * 512 + 512)
                                nc.vector.tensor_tensor(
                                    pT[:, sl], pT[:, sl], mask_sb[:, u, :],
                                    op=OP.mult)
                    pTs.append((pT, kb0, kb1))
                for i, (pT, kb0, kb1) in enumerate(pTs):
                    for half, kb in ((0, kb0), (1, kb1)):
                        nc.tensor.matmul(
                            pctx[:], v_sb[:, kb, h, :],
                            pT[:, half * 512 : half * 512 + 512],
                            start=(i == 0 and half == 0),
                            stop=(i == npair - 1 and half == 1))
                den_sb = rdp.tile([HD, 512], f32, name="den_sb")
                nc.vector.tensor_copy(den_sb[:], pctx[HD : 2 * HD, :])
                rden = rdp.tile([HD, 512], f32, name="rden")
                nc.vector.reciprocal_approx_fast(rden[:], den_sb[:])
                nc.vector.tensor_tensor(
                    ctxT[po : po + HD, h // 2, sp * 512 : sp * 512 + 512],
                    pctx[0:HD, :], rden[:], op=OP.mult)
            # ---- output projection partials for this span + RS ----
            for tq in range(4):
                tt = sp * 4 + tq
                for fo in range(2):
                    pwo = psC.tile([P, 512], f32, name="pwo", tag="pctx")
                    for kk in range(2):
                        nc.tensor.matmul(
                            pwo[:], ctxT[:, kk, tt * P : tt * P + P],
                            wout_sb[:, kk, fo * 512 : fo * 512 + 512],
                            start=(kk == 0), stop=(kk == 1))
                    c_out = coutp.tile([P, 512], bf16, name="c_out")
                    if fo == 0:
                        nc.vector.tensor_copy(c_out[:], pwo[:])
                    else:
                        nc.scalar.copy(c_out[:], pwo[:])
                    nc.sync.dma_start(
                        rs_in[sp][tq * P : tq * P + P,
                                  fo * 512 : fo * 512 + 512], c_out[:])
            nc.gpsimd.collective_compute(
                "ReduceScatter", mybir.AluOpType.add,
                ins=[rs_in[sp].opt()], outs=[rs_out[sp].opt()],
                replica_groups=groups)
            # readback (strip 3's is emitted after the w2 DMAs so the w2
            # load isn't queued behind the RS#3 wait on the gpsimd ring)
            if sp < 3:
                nc.gpsimd.dma_start(rs_all[:, sp, :], rs_out[sp][:, :])

        attn_st.close()
        ctx_st.close()
        qv_st.close()
        mask_st.close()
        wo_st.close()

        # w2 prefetch: starts once attention SBUF is released. Emitted on
        # the gpsimd ring BEFORE the strip-3 readback so it isn't queued
        # behind the RS#3 completion wait.
        w2_pool = w_st.enter_context(tc.tile_pool(name="w2p", bufs=1))
        w2_sb = w2_pool.tile([P, NFF, D], bf16)
        for c in range(8):
            s = c * (NFF // 8)
            nc.gpsimd.dma_start(
                w2_sb[:, s : s + NFF // 8, :].rearrange("p a b -> p (a b)"),
                w2_p[:, s * D : (s + NFF // 8) * D])
        nc.gpsimd.dma_start(rs_all[:, 3, :], rs_out[3][:, :])

        # ------------- Stage D: residual + LN2 -> h2T -------------
        mlp_st = ExitStack()
        mlp_pool = mlp_st.enter_context(tc.tile_pool(name="mlp", bufs=1))
        m1T = mlp_pool.tile([P, NFF, 512], bf16)
        h2T = mlp_pool.tile([P, ND, 512], bf16)

        lnB_st = ExitStack()
        lnB = lnB_st.enter_context(tc.tile_pool(name="lnB", bufs=2))
        lnBs = lnB_st.enter_context(tc.tile_pool(name="lnBs", bufs=4))

        def resid_stats(sp):
            nc.vector.tensor_add(x_mid[:, sp, :], x_mid[:, sp, :],
                                 rs_all[:, sp, :])
            statsB = lnBs.tile([P, 2, 6], f32, name="statsB")
            xmg = x_mid[:, sp, :].rearrange("p (g d) -> p g d", g=2)
            nc.vector.bn_stats(statsB[:, 0, :], xmg[:, 0, :])
            nc.vector.bn_stats(statsB[:, 1, :], xmg[:, 1, :])
            nc.vector.bn_aggr(mvB[:, sp, :], statsB[:])

        def ln2_strip(sp, trans_pool, trans_tag):
            rstdB = lnBs.tile([P, 1], f32, name="rstdB")
            nc.scalar.activation(rstdB[:], mvB[:, sp, 1:2], AF.Sqrt,
                                 bias=eps_t[:])
            nc.vector.reciprocal(rstdB[:], rstdB[:])
            h2 = lnB.tile([P, D], bf16, name="h2")
            nc.vector.tensor_scalar(h2[:], x_mid[:, sp, :], mvB[:, sp, 0:1],
                                    rstdB[:], op0=OP.subtract, op1=OP.mult)
            for dd in range(ND):
                ptr = trans_pool.tile([P, P], bf16, name=f"ptrD{trans_tag}",
                                      tag=trans_tag)
                nc.tensor.transpose(ptr[:], h2[:, dd * P : dd * P + P],
                                    identb[:])
                nc.any.tensor_copy(h2T[:, dd, sp * P : sp * P + P], ptr[:])

        with tc.tile_pool(name="psD2", bufs=2, space="PSUM") as psD2:
            for sp in range(3):
                resid_stats(sp)
                ln2_strip(sp, psD2, "ptrD")

        # ---------------- Stage E: MLP ----------------
        psE_st = ExitStack()
        psM1 = psE_st.enter_context(
            tc.tile_pool(name="psM1", bufs=2, space="PSUM"))
        psM2 = psE_st.enter_context(
            tc.tile_pool(name="psM2", bufs=1, space="PSUM"))

        # w1 pass1 (tokens 0:384) software-pipelined one ff ahead of the
        # w2 pass1 accumulation so the gelu handoff never stalls the PE
        pw2 = [[psM2.tile([P, 512], f32, name=f"pw2_{tl}_{fo}")
                for fo in range(2)] for tl in range(3)]

        def w1p1(ff):
            pm1 = psM1.tile([P, 384], f32, name="pm1", tag="pm1")
            for kk in range(ND):
                nc.tensor.matmul(pm1[:], w1_sb[:, ff, kk, :],
                                 h2T[:, kk, 0:384],
                                 start=(kk == 0), stop=(kk == ND - 1))
            nc.scalar.activation(m1T[:, ff, 0:384], pm1[:], AF.Gelu,
                                 bias=b1m[:, ff : ff + 1])

        def w2p1(ff):
            for tl in range(3):
                for fo in range(2):
                    nc.tensor.matmul(
                        pw2[tl][fo][:], m1T[:, ff, tl * P : tl * P + P],
                        w2_sb[:, ff, fo * 512 : fo * 512 + 512],
                        start=(ff == 0), stop=(ff == NFF - 1))

        w1p1(0)
        for ff in range(1, NFF):
            w1p1(ff)
            w2p1(ff - 1)
        w2p1(NFF - 1)
        # strip 3 residual+LN2 (waits on RS#3); emitted after the pass-1
        # loops so its Act/DVE work rides behind them in the queues
        resid_stats(3)
        ln2_strip(3, psM1, "pm1")

        with tc.tile_pool(name="outp", bufs=2) as outp:
            # finish + store strips 0-2
            for tl in range(3):
                for fo in range(2):
                    sl = slice(fo * 512, fo * 512 + 512)
                    o_sb = outp.tile([P, 512], f32, name="o_sb")
                    nc.vector.tensor_add(o_sb[:], pw2[tl][fo][:],
                                         x_mid[:, tl, sl])
                    nc.vector.tensor_add(o_sb[:], o_sb[:], b2_bc[:, sl])
                    nc.sync.dma_start(
                        out_strips[tl * P : tl * P + P, sl], o_sb[:])
            # strip 3: w1 pass2 pipelined one ff ahead of w2 pass2; w2
            # accumulators reuse the released pw2[0] slots
            pw2b = [psM2.tile([P, 512], f32, name=f"pw2b_{fo}",
                              tag=f"pw2_0_{fo}") for fo in range(2)]

            def w1p2(ff):
                pm1b = psM1.tile([P, P], f32, name="pm1b", tag="pm1")
                for kk in range(ND):
                    nc.tensor.matmul(pm1b[:], w1_sb[:, ff, kk, :],
                                     h2T[:, kk, 384:512],
                                     start=(kk == 0), stop=(kk == ND - 1))
                nc.scalar.activation(m1T[:, ff, 384:512], pm1b[:], AF.Gelu,
                                     bias=b1m[:, ff : ff + 1])

            def w2p2(ff):
                for fo in range(2):
                    nc.tensor.matmul(
                        pw2b[fo][:], m1T[:, ff, 384:512],
                        w2_sb[:, ff, fo * 512 : fo * 512 + 512],
                        start=(ff == 0), stop=(ff == NFF - 1))

            w1p2(0)
            for ff in range(1, NFF):
                w1p2(ff)
                w2p2(ff - 1)
            w2p2(NFF - 1)
            for fo in range(2):
                sl = slice(fo * 512, fo * 512 + 512)
                o_sb = outp.tile([P, 512], f32, name="o_sb")
                nc.vector.tensor_add(o_sb[:], pw2b[fo][:], x_mid[:, 3, sl])
                nc.vector.tensor_add(o_sb[:], o_sb[:], b2_bc[:, sl])
                nc.sync.dma_start(out_strips[3 * P : 4 * P, sl], o_sb[:])

        psE_st.close()
        lnB_st.close()
        mlp_st.close()
        w_st.close()
        dram_st.close()

    nc.compile()
    return nc


def _prepare_inputs(inputs):
    import ml_dtypes

    f64 = np.float64
    x = np.asarray(inputs["x"], dtype=np.float32)
    attn_mask = np.asarray(inputs["attn_mask"])
    ln1_g = np.asarray(inputs["ln1_g"], f64)
    ln1_b = np.asarray(inputs["ln1_b"], f64)
    ln2_g = np.asarray(inputs["ln2_g"], f64)
    ln2_b = np.asarray(inputs["ln2_b"], f64)
    wqkv = np.asarray(inputs["wqkv"], f64)
    bqkv = np.asarray(inputs["bqkv"], f64)
    wout = np.asarray(inputs["wout"], f64)
    bout = np.asarray(inputs["bout"], f64)
    w1 = np.asarray(inputs["w1"], f64)
    b1 = np.asarray(inputs["b1"], f64)
    w2 = np.asarray(inputs["w2"], f64)
    b2 = np.asarray(inputs["b2"], f64)

    bf = ml_dtypes.bfloat16

    wqkv_eff = ln1_g[:, None] * wqkv
    bqkv_eff = ln1_b @ wqkv + bqkv
    w1_eff = ln2_g[:, None] * w1
    b1_eff = ln2_b @ w1 + b1
    bvout = bqkv_eff[2 * D : 3 * D] @ wout + bout

    _, uniq, _ = _mask_structure(attn_mask)
    mask_u = (np.stack(uniq).reshape(-1, 512) if uniq
              else np.zeros((P, 512), np.float32))

    def pmajor3(w):  # [o*128, cols] -> [128, o*cols] partition-major
        o = w.shape[0] // P
        return np.ascontiguousarray(
            w.reshape(o, P, -1).transpose(1, 0, 2).reshape(P, -1))

    w1_prep = np.ascontiguousarray(
        w1_eff.astype(bf).reshape(ND, P, NFF, P).transpose(1, 2, 0, 3)
        .reshape(P, -1))
    w2_prep = np.ascontiguousarray(
        w2.astype(bf).reshape(NFF, P, D).transpose(1, 0, 2).reshape(P, -1))

    shared = {
        "mask_u": np.ascontiguousarray(mask_u.astype(bf)),
        "w1_p": w1_prep,
        "b1_eff": b1_eff.astype(np.float32),
        "w2_p": w2_prep,
        "b2": b2.astype(np.float32),
    }
    in_maps = []
    for c in range(NCORES):
        b, r = divmod(c, GROUP)
        hs = slice(r * HLOC * HD, (r + 1) * HLOC * HD)  # 256 features
        wq = wqkv_eff[:, 0:D][:, hs]
        wk = wqkv_eff[:, D : 2 * D][:, hs]
        wv = wqkv_eff[:, 2 * D : 3 * D][:, hs]
        wqkv_core = np.concatenate([wq, wk, wv], axis=1).astype(bf)
        wout_core = wout[hs, :].astype(bf)  # [256, D]
        strips = np.stack([
            x[b, sp * 512 + r * P : sp * 512 + (r + 1) * P, :]
            for sp in range(NSP)]).astype(f64) + bvout[None, None, :]
        in_maps.append(dict(
            shared,
            x_b=np.ascontiguousarray(x[b]),
            x_strips=np.ascontiguousarray(
                strips.reshape(4 * P, D).astype(np.float32)),
            wqkv_p=pmajor3(wqkv_core),
            bqk_s=np.ascontiguousarray(np.concatenate(
                [bqkv_eff[0:D][hs], bqkv_eff[D : 2 * D][hs]])
                .astype(np.float32)),
            wout_p=pmajor3(wout_core),
        ))
    return in_maps


def run(inputs, trace=False):
    from concourse.bass_utils import run_bass_kernel_spmd

    attn_mask = np.asarray(inputs["attn_mask"])
    cls, uniq, umap = _mask_structure(attn_mask)
    key = (tuple(sorted(cls.items())), tuple(sorted(umap.items())), len(uniq))
    if key not in _cache:
        _cache[key] = _build(key)
    nc = _cache[key]
    in_maps = _prepare_inputs(inputs)
    kw = {}
    if trace:
        kw = dict(trace=True, trace_cores=list(range(NCORES)))
    res = run_bass_kernel_spmd(nc, in_maps, core_ids=list(range(NCORES)), **kw)
    out = np.empty((B, T, D), np.float32)
    for c in range(NCORES):
        b, r = divmod(c, GROUP)
        strips = res.results[c]["out_strips"].reshape(NSP, P, D)
        for sp in range(NSP):
            out[b, sp * 512 + r * P : sp * 512 + (r + 1) * P, :] = strips[sp]
    return out, res


def kernel(**inputs):
    out, _ = run(inputs, trace=False)
    return out


# revision 53
# speedup vs baseline: 1.1281x; 1.0110x over previous
"""Trainium2 Bass kernel for a dense transformer block (B=2, T=2048, D=1024,
H=16, hd=64, MLP=4x), distributed across 8 NeuronCores.

Sharding: data-parallel over batch (cores 0-3 = batch 0, cores 4-7 = batch 1)
x tensor-parallel over heads (4 heads/core) for QKV+attention. The attention
output projection runs head-sharded per span (512 queries) and is re-sharded
token-parallel via one small bf16 ReduceScatter per span, pipelined behind the
following spans' attention compute, so the collective is (almost) never
exposed. Each core finishes LN2 + the MLP for the 4x128 token strips its
group's ReduceScatters deliver.

Performance structure (vs the naive version):
- All matmul operands are bf16; weights are cast+pre-transposed on the host so
  every weight DMA is contiguous per partition.
- LN scale/shift and biases are folded into the weight matrices host-side
  (wqkv <- g1*wqkv etc.), so on-chip LN is just (x-m)*rstd.
- Softmax denominators come from 64 replicated ones-columns appended to V:
  the PV matmul leaves ctx^T on partitions 0-63 and the denominator
  replicated on partitions 64-127, so normalization is one reciprocal + one
  multiply on the vector engine - no PE transposes in the softmax path.
- exp() runs on [128,1024] score-pair tiles to halve activation-engine
  instruction overhead; causal masking is a vector add of resident mask
  tiles (deduped across spans on the host).
- The tensor engine stream is kept dense (p-state ramp: 1.2GHz->2.4GHz after
  3us of continuous work): deep PSUM buffering, weights prefetched during
  attention, the last token strip's MLP work rides behind the final
  ReduceScatter while the other strips' MLP runs.
"""

from contextlib import ExitStack

import numpy as np

P = 128
B, T, D, HD = 2, 2048, 1024, 64
H = 16
MLPD = 4096
EPS = 1e-5
NCORES = 8
GROUP = 4          # cores per batch group
HLOC = H // GROUP  # heads per core
NT = T // P        # 16 token tiles
ND = D // P        # 8 feature tiles
NSP = T // 512     # 4 query spans
NFF = MLPD // P    # 32
SCALE = 1.0 / float(np.sqrt(HD))

_cache = {}


def _mask_structure(attn_mask):
    """Classify [span, kblock] and dedup the partial-block mask tiles.

    Returns (cls, uniq, umap):
      cls[(sp,kb)] in {"skip","free","partial"}
      uniq: list of np.ndarray [128,512] f32 additive masks (0 / -1e30)
      umap[(sp,kb)] = index into uniq for partial blocks
    """
    cls, uniq, umap, keys = {}, [], {}, {}
    m = np.asarray(attn_mask)
    for sp in range(NSP):
        qs = slice(sp * 512, sp * 512 + 512)
        for kb in range(NT):
            blk = m[qs, kb * P : kb * P + P]
            if blk.all():
                cls[(sp, kb)] = "skip"
            elif not blk.any():
                cls[(sp, kb)] = "free"
            else:
                cls[(sp, kb)] = "partial"
                add = np.where(blk.T, np.float32(0.0), np.float32(1.0))
                key = add.tobytes()
                if key not in keys:
                    keys[key] = len(uniq)
                    uniq.append(np.ascontiguousarray(add))
                umap[(sp, kb)] = keys[key]
    return cls, uniq, umap


def _build(mask_key):
    import concourse.bass as bass
    import concourse.mybir as mybir
    import concourse.tile as tile
    from concourse import bacc
    from concourse.masks import make_identity

    cls = dict(mask_key[0])
    umap = dict(mask_key[1])
    nuniq = mask_key[2]

    f32 = mybir.dt.float32
    bf16 = mybir.dt.bfloat16
    AF = mybir.ActivationFunctionType
    OP = mybir.AluOpType

    nc = bacc.Bacc("TRN2", target_bir_lowering=False, debug=False,
                   num_devices=NCORES)

    x_b = nc.dram_tensor("x_b", [T, D], f32, kind="ExternalInput")
    x_strips = nc.dram_tensor("x_strips", [4 * P, D], f32,
                              kind="ExternalInput")
    wqkv_p = nc.dram_tensor("wqkv_p", [P, ND * 768], bf16,
                            kind="ExternalInput")
    bqk_s = nc.dram_tensor("bqk_s", [512], f32, kind="ExternalInput")
    wout_p = nc.dram_tensor("wout_p", [P, 2 * D], bf16, kind="ExternalInput")
    mask_u = nc.dram_tensor("mask_u", [max(nuniq, 1) * P, 512], bf16,
                            kind="ExternalInput")
    w1_p = nc.dram_tensor("w1_p", [P, NFF * D], bf16, kind="ExternalInput")
    b1_eff = nc.dram_tensor("b1_eff", [MLPD], f32, kind="ExternalInput")
    w2_p = nc.dram_tensor("w2_p", [P, NFF * D], bf16, kind="ExternalInput")
    b2 = nc.dram_tensor("b2", [D], f32, kind="ExternalInput")
    out_strips = nc.dram_tensor("out_strips", [4 * P, D], f32,
                                kind="ExternalOutput")

    groups = [[0, 1, 2, 3], [4, 5, 6, 7]]

    def bcast_ap(handle, n):
        a = handle.ap()
        return bass.AP(tensor=a.tensor, offset=a.offset, ap=[[0, P], [1, n]])

    with tile.TileContext(nc) as tc, ExitStack() as st:
        consts = st.enter_context(tc.tile_pool(name="consts", bufs=1))
        identb = consts.tile([P, P], bf16)
        make_identity(nc, identb)
        eps_t = consts.tile([P, 1], f32)
        nc.vector.memset(eps_t[:], EPS)
        bqkt = consts.tile([P, 4], f32, name="bqkt")
        nc.gpsimd.dma_start(bqkt[:], bqk_s.ap().rearrange("(o p) -> p o", p=P))
        b1m = consts.tile([P, NFF], f32, name="b1m")
        nc.gpsimd.dma_start(b1m[:], b1_eff.ap().rearrange("(o p) -> p o", p=P))
        b2_bc = consts.tile([P, D], f32, name="b2bc")
        nc.gpsimd.dma_start(b2_bc[:], bcast_ap(b2, D))

        # Persistent state for stage D (residual + LN2 stats per strip).
        # x_mid is preloaded with x_strips (+bvout, host-folded); the
        # per-span ReduceScatter outputs land in rs_all and are added in.
        d_pool = st.enter_context(tc.tile_pool(name="dpool", bufs=1))
        x_mid = d_pool.tile([P, 4, D], f32)
        rs_all = d_pool.tile([P, 4, D], bf16)
        mvB = d_pool.tile([P, 4, 2], f32)

        # DRAM scratch for the per-span reduce-scatters
        dram_st = ExitStack()
        dram = dram_st.enter_context(tc.tile_pool(name="dram", bufs=1,
                                                  space="DRAM"))
        rs_in = [dram.tile([512, D], bf16, name=f"rsi{sp}")
                 for sp in range(NSP)]
        rs_out = [dram.tile([P, D], bf16, name=f"rso{sp}")
                  for sp in range(NSP)]

        # w1 + wout SBUF reserved up-front (pool stack is LIFO and they
        # outlive the attention-phase pools); their DMAs are emitted later.
        w_st = ExitStack()
        w1_pool = w_st.enter_context(tc.tile_pool(name="w1p", bufs=1))
        w1_sb = w1_pool.tile([P, NFF, ND, P], bf16)
        wo_st = ExitStack()
        wo_pool = wo_st.enter_context(tc.tile_pool(name="wop", bufs=1))
        wout_sb = wo_pool.tile([P, 2, D], bf16)

        mask_st = ExitStack()
        mask_pool = mask_st.enter_context(tc.tile_pool(name="maskp", bufs=1))
        mask_sb = mask_pool.tile([P, max(nuniq, 1), 512], bf16, name="msk")
        nc.gpsimd.dma_start(
            mask_sb[:], mask_u.ap().rearrange("(u p) q -> p u q", p=P))

        qv_st = ExitStack()
        qv_pool = qv_st.enter_context(tc.tile_pool(name="qv", bufs=1))
        qkT = qv_pool.tile([P, 4, T], bf16)
        v_sb = qv_pool.tile([P, NT, HLOC, 2 * HD], bf16)
        nc.gpsimd.memset(v_sb[:, :, :, HD : 2 * HD], 1.0)

        ctx_st = ExitStack()
        ctx_pool = ctx_st.enter_context(tc.tile_pool(name="ctxp", bufs=1))
        ctxT = ctx_pool.tile([P, 2, T], bf16)

        # ---------------- Stage A: LN1 + transpose -> hT ----------------
        ab_st = ExitStack()
        hT_pool = ab_st.enter_context(tc.tile_pool(name="hT", bufs=1))
        hT = hT_pool.tile([P, ND, T], bf16)
        wq_pool = ab_st.enter_context(tc.tile_pool(name="wqp", bufs=1))
        wqkv_sb = wq_pool.tile([P, ND, 768], bf16)
        nc.gpsimd.dma_start(wqkv_sb[:].rearrange("p a b -> p (a b)"),
                            wqkv_p[:, :])

        with tc.tile_pool(name="lnA", bufs=3) as lnA, \
             tc.tile_pool(name="lnAs", bufs=4) as lnAs, \
             tc.tile_pool(name="psA", bufs=3, space="PSUM") as psA, \
             tc.tile_pool(name="psB", bufs=4, space="PSUM") as psB:
            for nb in range(4):
                for tq in range(4):
                    tt = nb * 4 + tq
                    x_sb = lnA.tile([P, D], f32, name="x_sb")
                    nc.gpsimd.dma_start(x_sb[:], x_b[tt * P : tt * P + P, :])
                    stats = lnAs.tile([P, 2, 6], f32, name="stats")
                    xg = x_sb[:].rearrange("p (g d) -> p g d", g=2)
                    nc.vector.bn_stats(stats[:, 0, :], xg[:, 0, :])
                    nc.vector.bn_stats(stats[:, 1, :], xg[:, 1, :])
                    mv = lnAs.tile([P, 2], f32, name="mv")
                    nc.vector.bn_aggr(mv[:], stats[:])
                    rstd = lnAs.tile([P, 1], f32, name="rstd")
                    nc.scalar.activation(rstd[:], mv[:, 1:2], AF.Sqrt,
                                         bias=eps_t[:])
                    nc.vector.reciprocal(rstd[:], rstd[:])
                    negmr = lnAs.tile([P, 1], f32, name="negmr")
                    nc.vector.tensor_scalar(negmr[:], mv[:, 0:1], rstd[:],
                                            -1.0, op0=OP.mult, op1=OP.mult)
                    hno = lnA.tile([P, D], bf16, name="hno")
                    nc.scalar.activation(hno[:], x_sb[:], AF.Identity,
                                         bias=negmr[:], scale=rstd[:])
                    # all 8 transposes of this tile fill one PSUM bank,
                    # drained by a single wide copy (keeps the PE stream
                    # dense - no per-transpose slot round trips)
                    ptr8 = psA.tile([P, ND, P], bf16, name="ptr8")
                    for dd in range(ND):
                        nc.tensor.transpose(ptr8[:, dd, :],
                                            hno[:, dd * P : dd * P + P],
                                            identb[:])
                    nc.vector.tensor_copy(hT[:, :, tt * P : tt * P + P],
                                          ptr8[:])
                # -------- Stage B for this group of 4 token tiles --------
                for ft in range(4):
                    pq = psB.tile([P, 512], f32, name="pq")
                    for kk in range(ND):
                        nc.tensor.matmul(
                            pq[:], wqkv_sb[:, kk, ft * P : ft * P + P],
                            hT[:, kk, nb * 512 : nb * 512 + 512],
                            start=(kk == 0), stop=(kk == ND - 1))
                    nc.scalar.activation(
                        qkT[:, ft, nb * 512 : nb * 512 + 512], pq[:],
                        AF.Identity, bias=bqkt[:, ft : ft + 1])
                for tq in range(4):
                    tt = nb * 4 + tq
                    pv = psB.tile([P, 256], f32, name="pv", tag="pq")
                    for kk in range(ND):
                        nc.tensor.matmul(
                            pv[:], hT[:, kk, tt * P : tt * P + P],
                            wqkv_sb[:, kk, 512:768],
                            start=(kk == 0), stop=(kk == ND - 1))
                    nc.any.tensor_copy(
                        v_sb[:, tt, :, 0:HD],
                        pv[:].rearrange("p (h d) -> p h d", h=HLOC))

        ab_st.close()  # frees hT + wqkv (4MB + 1.5MB)

        # Weight DMAs for the next phases: wout now, w1 during attention.
        nc.gpsimd.dma_start(wout_sb[:].rearrange("p a b -> p (a b)"),
                            wout_p[:, :])
        for c in range(4):
            s = c * (NFF // 4)
            nc.gpsimd.dma_start(
                w1_sb[:, s : s + NFF // 4, :, :].rearrange(
                    "p a b c -> p (a b c)"),
                w1_p[:, s * D : (s + NFF // 4) * D])
        # Residual bases (x strips + folded bvout) preloaded into x_mid
        for sp in range(NSP):
            nc.gpsimd.dma_start(x_mid[:, sp, :],
                                x_strips[sp * P : sp * P + P, :])

        # ---------------- Stage C: attention ----------------
        attn_st = ExitStack()
        pTp = attn_st.enter_context(tc.tile_pool(name="pTp", bufs=10))
        rdp = attn_st.enter_context(tc.tile_pool(name="rdp", bufs=2))
        coutp = attn_st.enter_context(tc.tile_pool(name="coutp", bufs=6))
        psS = attn_st.enter_context(
            tc.tile_pool(name="psS", bufs=3, space="PSUM"))
        psC = attn_st.enter_context(
            tc.tile_pool(name="psC", bufs=2, space="PSUM"))

        for sp in range(NSP):
            kbs = [kb for kb in range(NT) if cls[(sp, kb)] != "skip"]
            assert len(kbs) % 2 == 0
            npair = len(kbs) // 2
            for h in range(HLOC):
                po = HD * (h % 2)
                pctx = psC.tile([P, 512], f32, name="pctx")
                # scores burst: all pairs' QK^T + exp first (deep pT
                # buffering), then a dense PV accumulation burst - the PE
                # never idles pair-by-pair waiting on the activation engine
                pTs = []
                for i in range(npair):
                    kb0, kb1 = kbs[2 * i], kbs[2 * i + 1]
                    pp = psS.tile([P, 1024], f32, name="pp")
                    for half, kb in ((0, kb0), (1, kb1)):
                        nc.tensor.matmul(
                            pp[:, half * 512 : half * 512 + 512],
                            qkT[po : po + HD, 2 + h // 2, kb * P : kb * P + P],
                            qkT[po : po + HD, h // 2,
                                sp * 512 : sp * 512 + 512],
                            start=True, stop=True)
                    pT = pTp.tile([P, 1024], bf16, name="pT")
                    nc.scalar.activation(pT[:], pp[:], AF.Exp, scale=SCALE)
                    # causal masking: multiply by resident 0/1 bf16 tiles
                    # (zeroed entries drop out of both ctx and denominator)
                    u0 = umap.get((sp, kb0))
                    u1 = umap.get((sp, kb1))
                    if u0 is not None and u1 == u0 + 1:
                        nc.vector.tensor_tensor(
                            pT[:], pT[:],
                            mask_sb[:, u0 : u0 + 2, :].rearrange(
                                "p a b -> p (a b)"), op=OP.mult)
                    else:
                        for half, u in ((0, u0), (1, u1)):
                            if u is not None:
                                sl = slice(half * 512, half * 512 + 512)
                                nc.vector.tensor_tensor(
                                    pT[:, sl], pT[:, sl], mask_sb[:, u, :],
                                    op=OP.mult)
                    pTs.append((pT, kb0, kb1))
                for i, (pT, kb0, kb1) in enumerate(pTs):
                    for half, kb in ((0, kb0), (1, kb1)):
                        nc.tensor.matmul(
                            pctx[:], v_sb[:, kb, h, :],
                            pT[:, half * 512 : half * 512 + 512],
                            start=(i == 0 and half == 0),
                            stop=(i == npair - 1 and half == 1))
                den_sb = rdp.tile([HD, 512], f32, name="den_sb")
                nc.vector.tensor_copy(den_sb[:], pctx[HD : 2 * HD, :])
                rden = rdp.tile([HD, 512], f32, name="rden")
                nc.vector.reciprocal_approx_fast(rden[:], den_sb[:])
                nc.vector.tensor_tensor(
                    ctxT[po : po + HD, h // 2, sp * 512 : sp * 512 + 512],
                    pctx[0:HD, :], rden[:], op=OP.mult)
            # ---- output projection partials for this span + RS ----
            for tq in range(4):
                tt = sp * 4 + tq
                for fo in range(2):
                    pwo = psC.tile([P, 512], f32, name="pwo", tag="pctx")
                    for kk in range(2):
                        nc.tensor.matmul(
                            pwo[:], ctxT[:, kk, tt * P : tt * P + P],
                            wout_sb[:, kk, fo * 512 : fo * 512 + 512],
                            start=(kk == 0), stop=(kk == 1))
                    c_out = coutp.tile([P, 512], bf16, name="c_out")
                    if fo == 0:
                        nc.vector.tensor_copy(c_out[:], pwo[:])
                    else:
                        nc.scalar.copy(c_out[:], pwo[:])
                    nc.sync.dma_start(
                        rs_in[sp][tq * P : tq * P + P,
                                  fo * 512 : fo * 512 + 512], c_out[:])
            nc.gpsimd.collective_compute(
                "ReduceScatter", mybir.AluOpType.add,
                ins=[rs_in[sp].opt()], outs=[rs_out[sp].opt()],
                replica_groups=groups)
            # readback (strip 3's is emitted after the w2 DMAs so the w2
            # load isn't queued behind the RS#3 wait on the gpsimd ring);
            # the residual add runs on the idle gpsimd engine, naturally
            # serialized behind its readback in the same queue - keeping
            # the RS-latency wait out of the DVE/PE streams
            if sp < 3:
                nc.gpsimd.dma_start(rs_all[:, sp, :], rs_out[sp][:, :])
                nc.gpsimd.tensor_add(x_mid[:, sp, :], x_mid[:, sp, :],
                                     rs_all[:, sp, :])

        attn_st.close()
        ctx_st.close()
        qv_st.close()
        mask_st.close()
        wo_st.close()

        # w2 prefetch: starts once attention SBUF is released. Emitted on
        # the gpsimd ring BEFORE the strip-3 readback so it isn't queued
        # behind the RS#3 completion wait.
        w2_pool = w_st.enter_context(tc.tile_pool(name="w2p", bufs=1))
        w2_sb = w2_pool.tile([P, NFF, D], bf16)
        for c in range(8):
            s = c * (NFF // 8)
            nc.gpsimd.dma_start(
                w2_sb[:, s : s + NFF // 8, :].rearrange("p a b -> p (a b)"),
                w2_p[:, s * D : (s + NFF // 8) * D])
        nc.gpsimd.dma_start(rs_all[:, 3, :], rs_out[3][:, :])
        nc.gpsimd.tensor_add(x_mid[:, 3, :], x_mid[:, 3, :], rs_all[:, 3, :])

        # ------------- Stage D: residual + LN2 -> h2T -------------
        mlp_st = ExitStack()
        mlp_pool = mlp_st.enter_context(tc.tile_pool(name="mlp", bufs=1))
        m1T = mlp_pool.tile([P, NFF, 512], bf16)
        h2T = mlp_pool.tile([P, ND, 512], bf16)

        lnB_st = ExitStack()
        lnB = lnB_st.enter_context(tc.tile_pool(name="lnB", bufs=2))
        lnBs = lnB_st.enter_context(tc.tile_pool(name="lnBs", bufs=4))

        def resid_stats(sp):
            statsB = lnBs.tile([P, 2, 6], f32, name="statsB")
            xmg = x_mid[:, sp, :].rearrange("p (g d) -> p g d", g=2)
            nc.vector.bn_stats(statsB[:, 0, :], xmg[:, 0, :])
            nc.vector.bn_stats(statsB[:, 1, :], xmg[:, 1, :])
            nc.vector.bn_aggr(mvB[:, sp, :], statsB[:])

        def ln2_strip(sp, trans_pool, trans_tag):
            rstdB = lnBs.tile([P, 1], f32, name="rstdB")
            nc.scalar.activation(rstdB[:], mvB[:, sp, 1:2], AF.Sqrt,
                                 bias=eps_t[:])
            nc.vector.reciprocal(rstdB[:], rstdB[:])
            h2 = lnB.tile([P, D], bf16, name="h2")
            nc.vector.tensor_scalar(h2[:], x_mid[:, sp, :], mvB[:, sp, 0:1],
                                    rstdB[:], op0=OP.subtract, op1=OP.mult)
            for dd in range(ND):
                ptr = trans_pool.tile([P, P], bf16, name=f"ptrD{trans_tag}",
                                      tag=trans_tag)
                nc.tensor.transpose(ptr[:], h2[:, dd * P : dd * P + P],
                                    identb[:])
                nc.any.tensor_copy(h2T[:, dd, sp * P : sp * P + P], ptr[:])

        with tc.tile_pool(name="psD2", bufs=2, space="PSUM") as psD2:
            for sp in range(3):
                resid_stats(sp)
                ln2_strip(sp, psD2, "ptrD")

        # ---------------- Stage E: MLP ----------------
        psE_st = ExitStack()
        psM1 = psE_st.enter_context(
            tc.tile_pool(name="psM1", bufs=2, space="PSUM"))
        psM2 = psE_st.enter_context(
            tc.tile_pool(name="psM2", bufs=1, space="PSUM"))

        # w1 pass1 (tokens 0:384) software-pipelined one ff ahead of the
        # w2 pass1 accumulation so the gelu handoff never stalls the PE
        pw2 = [[psM2.tile([P, 512], f32, name=f"pw2_{tl}_{fo}")
                for fo in range(2)] for tl in range(3)]

        def w1p1(ff):
            pm1 = psM1.tile([P, 384], f32, name="pm1", tag="pm1")
            for kk in range(ND):
                nc.tensor.matmul(pm1[:], w1_sb[:, ff, kk, :],
                                 h2T[:, kk, 0:384],
                                 start=(kk == 0), stop=(kk == ND - 1))
            nc.scalar.activation(m1T[:, ff, 0:384], pm1[:], AF.Gelu,
                                 bias=b1m[:, ff : ff + 1])

        def w2p1(ff):
            for tl in range(3):
                for fo in range(2):
                    nc.tensor.matmul(
                        pw2[tl][fo][:], m1T[:, ff, tl * P : tl * P + P],
                        w2_sb[:, ff, fo * 512 : fo * 512 + 512],
                        start=(ff == 0), stop=(ff == NFF - 1))

        w1p1(0)
        for ff in range(1, NFF):
            w1p1(ff)
            w2p1(ff - 1)
        w2p1(NFF - 1)
        # strip 3 residual+LN2 (waits on RS#3); emitted after the pass-1
        # loops so its Act/DVE work rides behind them in the queues
        resid_stats(3)
        ln2_strip(3, psM1, "pm1")

        with tc.tile_pool(name="outp", bufs=2) as outp:
            # finish + store strips 0-2
            for tl in range(3):
                for fo in range(2):
                    sl = slice(fo * 512, fo * 512 + 512)
                    o_sb = outp.tile([P, 512], f32, name="o_sb")
                    nc.vector.tensor_add(o_sb[:], pw2[tl][fo][:],
                                         x_mid[:, tl, sl])
                    nc.vector.tensor_add(o_sb[:], o_sb[:], b2_bc[:, sl])
                    nc.sync.dma_start(
                        out_strips[tl * P : tl * P + P, sl], o_sb[:])
            # strip 3: w1 pass2 pipelined one ff ahead of w2 pass2; w2
            # accumulators reuse the released pw2[0] slots
            pw2b = [psM2.tile([P, 512], f32, name=f"pw2b_{fo}",
                              tag=f"pw2_0_{fo}") for fo in range(2)]

            def w1p2(ff):
                pm1b = psM1.tile([P, P], f32, name="pm1b", tag="pm1")
                for kk in range(ND):
                    nc.tensor.matmul(pm1b[:], w1_sb[:, ff, kk, :],
                                     h2T[:, kk, 384:512],
                                     start=(kk == 0), stop=(kk == ND - 1))
                nc.scalar.activation(m1T[:, ff, 384:512], pm1b[:], AF.Gelu,
                                     bias=b1m[:, ff : ff + 1])

            def w2p2(ff):
                for fo in range(2):
                    nc.tensor.matmul(
                        pw2b[fo][:], m1T[:, ff, 384:512],
                        w2_sb[:, ff, fo * 512 : fo * 512 + 512],
                        start=(ff == 0), stop=(ff == NFF - 1))

            w1p2(0)
            for ff in range(1, NFF):
                w1p2(ff)
                w2p2(ff - 1)
            w2p2(NFF - 1)
            for fo in range(2):
                sl = slice(fo * 512, fo * 512 + 512)
                o_sb = outp.tile([P, 512], f32, name="o_sb")
                nc.vector.tensor_add(o_sb[:], pw2b[fo][:], x_mid[:, 3, sl])
                nc.vector.tensor_add(o_sb[:], o_sb[:], b2_bc[:, sl])
                nc.sync.dma_start(out_strips[3 * P : 4 * P, sl], o_sb[:])

        psE_st.close()
        lnB_st.close()
        mlp_st.close()
        w_st.close()
        dram_st.close()

    nc.compile()
    return nc


def _prepare_inputs(inputs):
    import ml_dtypes

    f64 = np.float64
    x = np.asarray(inputs["x"], dtype=np.float32)
    attn_mask = np.asarray(inputs["attn_mask"])
    ln1_g = np.asarray(inputs["ln1_g"], f64)
    ln1_b = np.asarray(inputs["ln1_b"], f64)
    ln2_g = np.asarray(inputs["ln2_g"], f64)
    ln2_b = np.asarray(inputs["ln2_b"], f64)
    wqkv = np.asarray(inputs["wqkv"], f64)
    bqkv = np.asarray(inputs["bqkv"], f64)
    wout = np.asarray(inputs["wout"], f64)
    bout = np.asarray(inputs["bout"], f64)
    w1 = np.asarray(inputs["w1"], f64)
    b1 = np.asarray(inputs["b1"], f64)
    w2 = np.asarray(inputs["w2"], f64)
    b2 = np.asarray(inputs["b2"], f64)

    bf = ml_dtypes.bfloat16

    wqkv_eff = ln1_g[:, None] * wqkv
    bqkv_eff = ln1_b @ wqkv + bqkv
    w1_eff = ln2_g[:, None] * w1
    b1_eff = ln2_b @ w1 + b1
    bvout = bqkv_eff[2 * D : 3 * D] @ wout + bout

    _, uniq, _ = _mask_structure(attn_mask)
    mask_u = (np.stack(uniq).reshape(-1, 512) if uniq
              else np.zeros((P, 512), np.float32))

    def pmajor3(w):  # [o*128, cols] -> [128, o*cols] partition-major
        o = w.shape[0] // P
        return np.ascontiguousarray(
            w.reshape(o, P, -1).transpose(1, 0, 2).reshape(P, -1))

    w1_prep = np.ascontiguousarray(
        w1_eff.astype(bf).reshape(ND, P, NFF, P).transpose(1, 2, 0, 3)
        .reshape(P, -1))
    w2_prep = np.ascontiguousarray(
        w2.astype(bf).reshape(NFF, P, D).transpose(1, 0, 2).reshape(P, -1))

    shared = {
        "mask_u": np.ascontiguousarray(mask_u.astype(bf)),
        "w1_p": w1_prep,
        "b1_eff": b1_eff.astype(np.float32),
        "w2_p": w2_prep,
        "b2": b2.astype(np.float32),
    }
    in_maps = []
    for c in range(NCORES):
        b, r = divmod(c, GROUP)
        hs = slice(r * HLOC * HD, (r + 1) * HLOC * HD)  # 256 features
        wq = wqkv_eff[:, 0:D][:, hs]
        wk = wqkv_eff[:, D : 2 * D][:, hs]
        wv = wqkv_eff[:, 2 * D : 3 * D][:, hs]
        wqkv_core = np.concatenate([wq, wk, wv], axis=1).astype(bf)
        wout_core = wout[hs, :].astype(bf)  # [256, D]
        strips = np.stack([
            x[b, sp * 512 + r * P : sp * 512 + (r + 1) * P, :]
            for sp in range(NSP)]).astype(f64) + bvout[None, None, :]
        in_maps.append(dict(
            shared,
            x_b=np.ascontiguousarray(x[b]),
            x_strips=np.ascontiguousarray(
                strips.reshape(4 * P, D).astype(np.float32)),
            wqkv_p=pmajor3(wqkv_core),
            bqk_s=np.ascontiguousarray(np.concatenate(
                [bqkv_eff[0:D][hs], bqkv_eff[D : 2 * D][hs]])
                .astype(np.float32)),
            wout_p=pmajor3(wout_core),
        ))
    return in_maps


def run(inputs, trace=False):
    from concourse.bass_utils import run_bass_kernel_spmd

    attn_mask = np.asarray(inputs["attn_mask"])
    cls, uniq, umap = _mask_structure(attn_mask)
    key = (tuple(sorted(cls.items())), tuple(sorted(umap.items())), len(uniq))
    if key not in _cache:
        _cache[key] = _build(key)
    nc = _cache[key]
    in_maps = _prepare_inputs(inputs)
    kw = {}
    if trace:
        kw = dict(trace=True, trace_cores=list(range(NCORES)))
    res = run_bass_kernel_spmd(nc, in_maps, core_ids=list(range(NCORES)), **kw)
    out = np.empty((B, T, D), np.float32)
    for c in range(NCORES):
        b, r = divmod(c, GROUP)
        strips = res.results[c]["out_strips"].reshape(NSP, P, D)
        for sp in range(NSP):
            out[b, sp * 512 + r * P : sp * 512 + (r + 1) * P, :] = strips[sp]
    return out, res


def kernel(**inputs):
    out, _ = run(inputs, trace=False)
    return out


# revision 55
# speedup vs baseline: 1.1398x; 1.0103x over previous
"""Trainium2 Bass kernel for a dense transformer block (B=2, T=2048, D=1024,
H=16, hd=64, MLP=4x), distributed across 8 NeuronCores.

Sharding: data-parallel over batch (cores 0-3 = batch 0, cores 4-7 = batch 1)
x tensor-parallel over heads (4 heads/core) for QKV+attention. The attention
output projection runs head-sharded per span (512 queries) and is re-sharded
token-parallel via one small bf16 ReduceScatter per span, pipelined behind the
following spans' attention compute, so the collective is (almost) never
exposed. Each core finishes LN2 + the MLP for the 4x128 token strips its
group's ReduceScatters deliver.

Performance structure (vs the naive version):
- All matmul operands are bf16; weights are cast+pre-transposed on the host so
  every weight DMA is contiguous per partition.
- LN scale/shift and biases are folded into the weight matrices host-side
  (wqkv <- g1*wqkv etc.), so on-chip LN is just (x-m)*rstd.
- Softmax denominators come from 64 replicated ones-columns appended to V:
  the PV matmul leaves ctx^T on partitions 0-63 and the denominator
  replicated on partitions 64-127, so normalization is one reciprocal + one
  multiply on the vector engine - no PE transposes in the softmax path.
- exp() runs on [128,1024] score-pair tiles to halve activation-engine
  instruction overhead; causal masking is a vector add of resident mask
  tiles (deduped across spans on the host).
- The tensor engine stream is kept dense (p-state ramp: 1.2GHz->2.4GHz after
  3us of continuous work): deep PSUM buffering, weights prefetched during
  attention, the last token strip's MLP work rides behind the final
  ReduceScatter while the other strips' MLP runs.
"""

from contextlib import ExitStack

import numpy as np

P = 128
B, T, D, HD = 2, 2048, 1024, 64
H = 16
MLPD = 4096
EPS = 1e-5
NCORES = 8
GROUP = 4          # cores per batch group
HLOC = H // GROUP  # heads per core
NT = T // P        # 16 token tiles
ND = D // P        # 8 feature tiles
NSP = T // 512     # 4 query spans
NFF = MLPD // P    # 32
SCALE = 1.0 / float(np.sqrt(HD))

_cache = {}


def _mask_structure(attn_mask):
    """Classify [span, kblock] and dedup the partial-block mask tiles.

    Returns (cls, uniq, umap):
      cls[(sp,kb)] in {"skip","free","partial"}
      uniq: list of np.ndarray [128,512] f32 additive masks (0 / -1e30)
      umap[(sp,kb)] = index into uniq for partial blocks
    """
    cls, uniq, umap, keys = {}, [], {}, {}
    m = np.asarray(attn_mask)
    for sp in range(NSP):
        qs = slice(sp * 512, sp * 512 + 512)
        for kb in range(NT):
            blk = m[qs, kb * P : kb * P + P]
            if blk.all():
                cls[(sp, kb)] = "skip"
            elif not blk.any():
                cls[(sp, kb)] = "free"
            else:
                cls[(sp, kb)] = "partial"
                add = np.where(blk.T, np.float32(0.0), np.float32(1.0))
                key = add.tobytes()
                if key not in keys:
                    keys[key] = len(uniq)
                    uniq.append(np.ascontiguousarray(add))
                umap[(sp, kb)] = keys[key]
    return cls, uniq, umap


def _build(mask_key):
    import concourse.bass as bass
    import concourse.mybir as mybir
    import concourse.tile as tile
    from concourse import bacc
    from concourse.masks import make_identity

    cls = dict(mask_key[0])
    umap = dict(mask_key[1])
    nuniq = mask_key[2]

    f32 = mybir.dt.float32
    bf16 = mybir.dt.bfloat16
    AF = mybir.ActivationFunctionType
    OP = mybir.AluOpType

    nc = bacc.Bacc("TRN2", target_bir_lowering=False, debug=False,
                   num_devices=NCORES)

    x_b = nc.dram_tensor("x_b", [T, D], f32, kind="ExternalInput")
    x_strips = nc.dram_tensor("x_strips", [4 * P, D], f32,
                              kind="ExternalInput")
    wqkv_p = nc.dram_tensor("wqkv_p", [P, ND * 768], bf16,
                            kind="ExternalInput")
    bqk_s = nc.dram_tensor("bqk_s", [512], f32, kind="ExternalInput")
    wout_p = nc.dram_tensor("wout_p", [P, 2 * D], bf16, kind="ExternalInput")
    mask_u = nc.dram_tensor("mask_u", [max(nuniq, 1) * P, 512], bf16,
                            kind="ExternalInput")
    w1_p = nc.dram_tensor("w1_p", [P, NFF * D], bf16, kind="ExternalInput")
    b1_eff = nc.dram_tensor("b1_eff", [MLPD], f32, kind="ExternalInput")
    w2_p = nc.dram_tensor("w2_p", [P, NFF * D], bf16, kind="ExternalInput")
    b2 = nc.dram_tensor("b2", [D], f32, kind="ExternalInput")
    out_strips = nc.dram_tensor("out_strips", [4 * P, D], f32,
                                kind="ExternalOutput")

    groups = [[0, 1, 2, 3], [4, 5, 6, 7]]

    def bcast_ap(handle, n):
        a = handle.ap()
        return bass.AP(tensor=a.tensor, offset=a.offset, ap=[[0, P], [1, n]])

    with tile.TileContext(nc) as tc, ExitStack() as st:
        consts = st.enter_context(tc.tile_pool(name="consts", bufs=1))
        identb = consts.tile([P, P], bf16)
        make_identity(nc, identb)
        eps_t = consts.tile([P, 1], f32)
        nc.vector.memset(eps_t[:], EPS)
        bqkt = consts.tile([P, 4], f32, name="bqkt")
        nc.gpsimd.dma_start(bqkt[:], bqk_s.ap().rearrange("(o p) -> p o", p=P))
        b1m = consts.tile([P, NFF], f32, name="b1m")
        nc.gpsimd.dma_start(b1m[:], b1_eff.ap().rearrange("(o p) -> p o", p=P))
        b2_bc = consts.tile([P, D], f32, name="b2bc")
        nc.gpsimd.dma_start(b2_bc[:], bcast_ap(b2, D))

        # Persistent state for stage D (residual + LN2 stats per strip).
        # x_mid is preloaded with x_strips (+bvout, host-folded); the
        # per-span ReduceScatter outputs land in rs_all and are added in.
        d_pool = st.enter_context(tc.tile_pool(name="dpool", bufs=1))
        x_mid = d_pool.tile([P, 4, D], f32)
        rs_all = d_pool.tile([P, 4, D], bf16)
        mvB = d_pool.tile([P, 4, 2], f32)

        # DRAM scratch for the per-span reduce-scatters
        dram_st = ExitStack()
        dram = dram_st.enter_context(tc.tile_pool(name="dram", bufs=1,
                                                  space="DRAM"))
        rs_in = [dram.tile([512, D], bf16, name=f"rsi{sp}")
                 for sp in range(NSP)]
        rs_out = [dram.tile([P, D], bf16, name=f"rso{sp}")
                  for sp in range(NSP)]

        # w1 + wout SBUF reserved up-front (pool stack is LIFO and they
        # outlive the attention-phase pools); their DMAs are emitted later.
        w_st = ExitStack()
        w1_pool = w_st.enter_context(tc.tile_pool(name="w1p", bufs=1))
        w1_sb = w1_pool.tile([P, NFF, ND, P], bf16)
        wo_st = ExitStack()
        wo_pool = wo_st.enter_context(tc.tile_pool(name="wop", bufs=1))
        wout_sb = wo_pool.tile([P, 2, D], bf16)

        mask_st = ExitStack()
        mask_pool = mask_st.enter_context(tc.tile_pool(name="maskp", bufs=1))
        mask_sb = mask_pool.tile([P, max(nuniq, 1), 512], bf16, name="msk")
        nc.gpsimd.dma_start(
            mask_sb[:], mask_u.ap().rearrange("(u p) q -> p u q", p=P))

        qv_st = ExitStack()
        qv_pool = qv_st.enter_context(tc.tile_pool(name="qv", bufs=1))
        qkT = qv_pool.tile([P, 4, T], bf16)
        v_sb = qv_pool.tile([P, NT, HLOC, 2 * HD], bf16)
        nc.gpsimd.memset(v_sb[:, :, :, HD : 2 * HD], 1.0)

        ctx_st = ExitStack()
        ctx_pool = ctx_st.enter_context(tc.tile_pool(name="ctxp", bufs=1))
        ctxT = ctx_pool.tile([P, 2, T], bf16)

        # ---------------- Stage A: LN1 + transpose -> hT ----------------
        ab_st = ExitStack()
        hT_pool = ab_st.enter_context(tc.tile_pool(name="hT", bufs=1))
        hT = hT_pool.tile([P, ND, T], bf16)
        wq_pool = ab_st.enter_context(tc.tile_pool(name="wqp", bufs=1))
        wqkv_sb = wq_pool.tile([P, ND, 768], bf16)
        nc.gpsimd.dma_start(wqkv_sb[:].rearrange("p a b -> p (a b)"),
                            wqkv_p[:, :])

        with tc.tile_pool(name="lnA", bufs=3) as lnA, \
             tc.tile_pool(name="lnAs", bufs=4) as lnAs, \
             tc.tile_pool(name="psA", bufs=3, space="PSUM") as psA, \
             tc.tile_pool(name="psB", bufs=4, space="PSUM") as psB:
            for nb in range(4):
                for tq in range(4):
                    tt = nb * 4 + tq
                    x_sb = lnA.tile([P, D], f32, name="x_sb")
                    nc.gpsimd.dma_start(x_sb[:], x_b[tt * P : tt * P + P, :])
                    stats = lnAs.tile([P, 2, 6], f32, name="stats")
                    xg = x_sb[:].rearrange("p (g d) -> p g d", g=2)
                    nc.vector.bn_stats(stats[:, 0, :], xg[:, 0, :])
                    nc.vector.bn_stats(stats[:, 1, :], xg[:, 1, :])
                    mv = lnAs.tile([P, 2], f32, name="mv")
                    nc.vector.bn_aggr(mv[:], stats[:])
                    rstd = lnAs.tile([P, 1], f32, name="rstd")
                    nc.scalar.activation(rstd[:], mv[:, 1:2], AF.Sqrt,
                                         bias=eps_t[:])
                    nc.vector.reciprocal(rstd[:], rstd[:])
                    negmr = lnAs.tile([P, 1], f32, name="negmr")
                    nc.vector.tensor_scalar(negmr[:], mv[:, 0:1], rstd[:],
                                            -1.0, op0=OP.mult, op1=OP.mult)
                    hno = lnA.tile([P, D], bf16, name="hno")
                    nc.scalar.activation(hno[:], x_sb[:], AF.Identity,
                                         bias=negmr[:], scale=rstd[:])
                    # all 8 transposes of this tile fill one PSUM bank,
                    # drained by a single wide copy (keeps the PE stream
                    # dense - no per-transpose slot round trips)
                    ptr8 = psA.tile([P, ND, P], bf16, name="ptr8")
                    for dd in range(ND):
                        nc.tensor.transpose(ptr8[:, dd, :],
                                            hno[:, dd * P : dd * P + P],
                                            identb[:])
                    nc.vector.tensor_copy(hT[:, :, tt * P : tt * P + P],
                                          ptr8[:])
                # -------- Stage B for this group of 4 token tiles --------
                for ft in range(4):
                    pq = psB.tile([P, 512], f32, name="pq")
                    for kk in range(ND):
                        nc.tensor.matmul(
                            pq[:], wqkv_sb[:, kk, ft * P : ft * P + P],
                            hT[:, kk, nb * 512 : nb * 512 + 512],
                            start=(kk == 0), stop=(kk == ND - 1))
                    nc.scalar.activation(
                        qkT[:, ft, nb * 512 : nb * 512 + 512], pq[:],
                        AF.Identity, bias=bqkt[:, ft : ft + 1])
                for tq in range(4):
                    tt = nb * 4 + tq
                    pv = psB.tile([P, 256], f32, name="pv", tag="pq")
                    for kk in range(ND):
                        nc.tensor.matmul(
                            pv[:], hT[:, kk, tt * P : tt * P + P],
                            wqkv_sb[:, kk, 512:768],
                            start=(kk == 0), stop=(kk == ND - 1))
                    nc.any.tensor_copy(
                        v_sb[:, tt, :, 0:HD],
                        pv[:].rearrange("p (h d) -> p h d", h=HLOC))

        ab_st.close()  # frees hT + wqkv (4MB + 1.5MB)

        # Weight DMAs for the next phases: wout now, w1 during attention.
        nc.gpsimd.dma_start(wout_sb[:].rearrange("p a b -> p (a b)"),
                            wout_p[:, :])
        for c in range(4):
            s = c * (NFF // 4)
            nc.gpsimd.dma_start(
                w1_sb[:, s : s + NFF // 4, :, :].rearrange(
                    "p a b c -> p (a b c)"),
                w1_p[:, s * D : (s + NFF // 4) * D])
        # Residual bases (x strips + folded bvout) preloaded into x_mid
        for sp in range(NSP):
            nc.gpsimd.dma_start(x_mid[:, sp, :],
                                x_strips[sp * P : sp * P + P, :])

        # ---------------- Stage C: attention ----------------
        attn_st = ExitStack()
        pTp = attn_st.enter_context(tc.tile_pool(name="pTp", bufs=10))
        rdp = attn_st.enter_context(tc.tile_pool(name="rdp", bufs=2))
        coutp = attn_st.enter_context(tc.tile_pool(name="coutp", bufs=6))
        psS = attn_st.enter_context(
            tc.tile_pool(name="psS", bufs=3, space="PSUM"))
        psC = attn_st.enter_context(
            tc.tile_pool(name="psC", bufs=2, space="PSUM"))

        for sp in range(NSP):
            kbs = [kb for kb in range(NT) if cls[(sp, kb)] != "skip"]
            assert len(kbs) % 2 == 0
            nquad = len(kbs) // 2
            for h in range(HLOC):
                po = HD * (h % 2)
                pctx = psC.tile([P, 512], f32, name="pctx")
                # scores burst: all quads' QK^T + one exp per [128,2048]
                # quad (minimizes activation-engine instruction overhead),
                # then a dense PV accumulation burst
                pTs = []
                for i in range(nquad):
                    qkbs = kbs[2 * i : 2 * i + 2]
                    pp = psS.tile([P, 1024], f32, name="pp")
                    for q, kb in enumerate(qkbs):
                        nc.tensor.matmul(
                            pp[:, q * 512 : q * 512 + 512],
                            qkT[po : po + HD, 2 + h // 2, kb * P : kb * P + P],
                            qkT[po : po + HD, h // 2,
                                sp * 512 : sp * 512 + 512],
                            start=True, stop=True)
                    pT = pTp.tile([P, 1024], bf16, name="pT")
                    nc.scalar.activation(pT[:], pp[:], AF.Exp, scale=SCALE)
                    # causal masking: multiply by resident 0/1 bf16 tiles
                    # (zeroed entries drop out of both ctx and denominator)
                    us = [umap.get((sp, kb)) for kb in qkbs]
                    if any(u is not None for u in us):
                        if (None not in us
                                and us == list(range(us[0], us[0] + 2))):
                            nc.vector.tensor_tensor(
                                pT[:], pT[:],
                                mask_sb[:, us[0] : us[0] + 2, :].rearrange(
                                    "p a b -> p (a b)"), op=OP.mult)
                        else:
                            for q, u in enumerate(us):
                                if u is not None:
                                    sl = slice(q * 512, q * 512 + 512)
                                    nc.vector.tensor_tensor(
                                        pT[:, sl], pT[:, sl],
                                        mask_sb[:, u, :], op=OP.mult)
                    pTs.append((pT, qkbs))
                for i, (pT, qkbs) in enumerate(pTs):
                    for q, kb in enumerate(qkbs):
                        nc.tensor.matmul(
                            pctx[:], v_sb[:, kb, h, :],
                            pT[:, q * 512 : q * 512 + 512],
                            start=(i == 0 and q == 0),
                            stop=(i == nquad - 1 and q == 1))
                den_sb = rdp.tile([HD, 512], f32, name="den_sb")
                nc.vector.tensor_copy(den_sb[:], pctx[HD : 2 * HD, :])
                rden = rdp.tile([HD, 512], f32, name="rden")
                nc.vector.reciprocal_approx_fast(rden[:], den_sb[:])
                nc.vector.tensor_tensor(
                    ctxT[po : po + HD, h // 2, sp * 512 : sp * 512 + 512],
                    pctx[0:HD, :], rden[:], op=OP.mult)
            # ---- output projection partials for this span + RS ----
            for tq in range(4):
                tt = sp * 4 + tq
                for fo in range(2):
                    pwo = psC.tile([P, 512], f32, name="pwo", tag="pctx")
                    for kk in range(2):
                        nc.tensor.matmul(
                            pwo[:], ctxT[:, kk, tt * P : tt * P + P],
                            wout_sb[:, kk, fo * 512 : fo * 512 + 512],
                            start=(kk == 0), stop=(kk == 1))
                    c_out = coutp.tile([P, 512], bf16, name="c_out")
                    nc.vector.tensor_copy(c_out[:], pwo[:])
                    nc.sync.dma_start(
                        rs_in[sp][tq * P : tq * P + P,
                                  fo * 512 : fo * 512 + 512], c_out[:])
            nc.gpsimd.collective_compute(
                "ReduceScatter", mybir.AluOpType.add,
                ins=[rs_in[sp].opt()], outs=[rs_out[sp].opt()],
                replica_groups=groups)
            # readback (strip 3's is emitted after the w2 DMAs so the w2
            # load isn't queued behind the RS#3 wait on the gpsimd ring);
            # the residual add runs on the idle gpsimd engine, naturally
            # serialized behind its readback in the same queue - keeping
            # the RS-latency wait out of the DVE/PE streams
            if sp < 3:
                nc.gpsimd.dma_start(rs_all[:, sp, :], rs_out[sp][:, :])
                nc.gpsimd.tensor_add(x_mid[:, sp, :], x_mid[:, sp, :],
                                     rs_all[:, sp, :])

        attn_st.close()
        ctx_st.close()
        qv_st.close()
        mask_st.close()
        wo_st.close()

        # w2 prefetch: starts once attention SBUF is released. Emitted on
        # the gpsimd ring BEFORE the strip-3 readback so it isn't queued
        # behind the RS#3 completion wait.
        w2_pool = w_st.enter_context(tc.tile_pool(name="w2p", bufs=1))
        w2_sb = w2_pool.tile([P, NFF, D], bf16)
        for c in range(8):
            s = c * (NFF // 8)
            nc.gpsimd.dma_start(
                w2_sb[:, s : s + NFF // 8, :].rearrange("p a b -> p (a b)"),
                w2_p[:, s * D : (s + NFF // 8) * D])
        nc.gpsimd.dma_start(rs_all[:, 3, :], rs_out[3][:, :])
        nc.gpsimd.tensor_add(x_mid[:, 3, :], x_mid[:, 3, :], rs_all[:, 3, :])

        # ------------- Stage D: residual + LN2 -> h2T -------------
        mlp_st = ExitStack()
        mlp_pool = mlp_st.enter_context(tc.tile_pool(name="mlp", bufs=1))
        m1T = mlp_pool.tile([P, NFF, 512], bf16)
        h2T = mlp_pool.tile([P, ND, 512], bf16)

        lnB_st = ExitStack()
        lnB = lnB_st.enter_context(tc.tile_pool(name="lnB", bufs=2))
        lnBs = lnB_st.enter_context(tc.tile_pool(name="lnBs", bufs=4))

        def resid_stats(sp):
            statsB = lnBs.tile([P, 2, 6], f32, name="statsB")
            xmg = x_mid[:, sp, :].rearrange("p (g d) -> p g d", g=2)
            nc.vector.bn_stats(statsB[:, 0, :], xmg[:, 0, :])
            nc.vector.bn_stats(statsB[:, 1, :], xmg[:, 1, :])
            nc.vector.bn_aggr(mvB[:, sp, :], statsB[:])

        def ln2_strip(sp, trans_pool, trans_tag):
            rstdB = lnBs.tile([P, 1], f32, name="rstdB")
            nc.scalar.activation(rstdB[:], mvB[:, sp, 1:2], AF.Sqrt,
                                 bias=eps_t[:])
            nc.vector.reciprocal(rstdB[:], rstdB[:])
            h2 = lnB.tile([P, D], bf16, name="h2")
            nc.vector.tensor_scalar(h2[:], x_mid[:, sp, :], mvB[:, sp, 0:1],
                                    rstdB[:], op0=OP.subtract, op1=OP.mult)
            for dd in range(ND):
                ptr = trans_pool.tile([P, P], bf16, name=f"ptrD{trans_tag}",
                                      tag=trans_tag)
                nc.tensor.transpose(ptr[:], h2[:, dd * P : dd * P + P],
                                    identb[:])
                nc.any.tensor_copy(h2T[:, dd, sp * P : sp * P + P], ptr[:])

        with tc.tile_pool(name="psD2", bufs=2, space="PSUM") as psD2:
            for sp in range(3):
                resid_stats(sp)
                ln2_strip(sp, psD2, "ptrD")

        # ---------------- Stage E: MLP ----------------
        psE_st = ExitStack()
        psM1 = psE_st.enter_context(
            tc.tile_pool(name="psM1", bufs=2, space="PSUM"))
        psM2 = psE_st.enter_context(
            tc.tile_pool(name="psM2", bufs=1, space="PSUM"))

        # w1 pass1 (tokens 0:384) software-pipelined one ff ahead of the
        # w2 pass1 accumulation so the gelu handoff never stalls the PE
        pw2 = [[psM2.tile([P, 512], f32, name=f"pw2_{tl}_{fo}")
                for fo in range(2)] for tl in range(3)]

        def w1p1(ff):
            pm1 = psM1.tile([P, 384], f32, name="pm1", tag="pm1")
            for kk in range(ND):
                nc.tensor.matmul(pm1[:], w1_sb[:, ff, kk, :],
                                 h2T[:, kk, 0:384],
                                 start=(kk == 0), stop=(kk == ND - 1))
            nc.scalar.activation(m1T[:, ff, 0:384], pm1[:], AF.Gelu,
                                 bias=b1m[:, ff : ff + 1])

        def w2p1(ff):
            for tl in range(3):
                for fo in range(2):
                    nc.tensor.matmul(
                        pw2[tl][fo][:], m1T[:, ff, tl * P : tl * P + P],
                        w2_sb[:, ff, fo * 512 : fo * 512 + 512],
                        start=(ff == 0), stop=(ff == NFF - 1))

        w1p1(0)
        for ff in range(1, NFF):
            w1p1(ff)
            w2p1(ff - 1)
        w2p1(NFF - 1)
        # strip 3 residual+LN2 (waits on RS#3); emitted after the pass-1
        # loops so its Act/DVE work rides behind them in the queues
        resid_stats(3)
        ln2_strip(3, psM1, "pm1")

        with tc.tile_pool(name="outp", bufs=2) as outp:
            # finish + store strips 0-2
            for tl in range(3):
                for fo in range(2):
                    sl = slice(fo * 512, fo * 512 + 512)
                    o_sb = outp.tile([P, 512], f32, name="o_sb")
                    nc.vector.tensor_add(o_sb[:], pw2[tl][fo][:],
                                         x_mid[:, tl, sl])
                    nc.vector.tensor_add(o_sb[:], o_sb[:], b2_bc[:, sl])
                    nc.sync.dma_start(
                        out_strips[tl * P : tl * P + P, sl], o_sb[:])
            # strip 3: w1 pass2 pipelined one ff ahead of w2 pass2; w2
            # accumulators reuse the released pw2[0] slots
            pw2b = [psM2.tile([P, 512], f32, name=f"pw2b_{fo}",
                              tag=f"pw2_0_{fo}") for fo in range(2)]

            def w1p2(ff):
                pm1b = psM1.tile([P, P], f32, name="pm1b", tag="pm1")
                for kk in range(ND):
                    nc.tensor.matmul(pm1b[:], w1_sb[:, ff, kk, :],
                                     h2T[:, kk, 384:512],
                                     start=(kk == 0), stop=(kk == ND - 1))
                nc.scalar.activation(m1T[:, ff, 384:512], pm1b[:], AF.Gelu,
                                     bias=b1m[:, ff : ff + 1])

            def w2p2(ff):
                for fo in range(2):
                    nc.tensor.matmul(
                        pw2b[fo][:], m1T[:, ff, 384:512],
                        w2_sb[:, ff, fo * 512 : fo * 512 + 512],
                        start=(ff == 0), stop=(ff == NFF - 1))

            w1p2(0)
            for ff in range(1, NFF):
                w1p2(ff)
                w2p2(ff - 1)
            w2p2(NFF - 1)
            for fo in range(2):
                sl = slice(fo * 512, fo * 512 + 512)
                o_sb = outp.tile([P, 512], f32, name="o_sb")
                nc.vector.tensor_add(o_sb[:], pw2b[fo][:], x_mid[:, 3, sl])
                nc.vector.tensor_add(o_sb[:], o_sb[:], b2_bc[:, sl])
                nc.sync.dma_start(out_strips[3 * P : 4 * P, sl], o_sb[:])

        psE_st.close()
        lnB_st.close()
        mlp_st.close()
        w_st.close()
        dram_st.close()

    nc.compile()
    return nc


def _prepare_inputs(inputs):
    import ml_dtypes

    f64 = np.float64
    x = np.asarray(inputs["x"], dtype=np.float32)
    attn_mask = np.asarray(inputs["attn_mask"])
    ln1_g = np.asarray(inputs["ln1_g"], f64)
    ln1_b = np.asarray(inputs["ln1_b"], f64)
    ln2_g = np.asarray(inputs["ln2_g"], f64)
    ln2_b = np.asarray(inputs["ln2_b"], f64)
    wqkv = np.asarray(inputs["wqkv"], f64)
    bqkv = np.asarray(inputs["bqkv"], f64)
    wout = np.asarray(inputs["wout"], f64)
    bout = np.asarray(inputs["bout"], f64)
    w1 = np.asarray(inputs["w1"], f64)
    b1 = np.asarray(inputs["b1"], f64)
    w2 = np.asarray(inputs["w2"], f64)
    b2 = np.asarray(inputs["b2"], f64)

    bf = ml_dtypes.bfloat16

    wqkv_eff = ln1_g[:, None] * wqkv
    bqkv_eff = ln1_b @ wqkv + bqkv
    w1_eff = ln2_g[:, None] * w1
    b1_eff = ln2_b @ w1 + b1
    bvout = bqkv_eff[2 * D : 3 * D] @ wout + bout

    _, uniq, _ = _mask_structure(attn_mask)
    mask_u = (np.stack(uniq).reshape(-1, 512) if uniq
              else np.zeros((P, 512), np.float32))

    def pmajor3(w):  # [o*128, cols] -> [128, o*cols] partition-major
        o = w.shape[0] // P
        return np.ascontiguousarray(
            w.reshape(o, P, -1).transpose(1, 0, 2).reshape(P, -1))

    w1_prep = np.ascontiguousarray(
        w1_eff.astype(bf).reshape(ND, P, NFF, P).transpose(1, 2, 0, 3)
        .reshape(P, -1))
    w2_prep = np.ascontiguousarray(
        w2.astype(bf).reshape(NFF, P, D).transpose(1, 0, 2).reshape(P, -1))

    shared = {
        "mask_u": np.ascontiguousarray(mask_u.astype(bf)),
        "w1_p": w1_prep,
        "b1_eff": b1_eff.astype(np.float32),
        "w2_p": w2_prep,
        "b2": b2.astype(np.float32),
    }
    in_maps = []
    for c in range(NCORES):
        b, r = divmod(c, GROUP)
        hs = slice(r * HLOC * HD, (r + 1) * HLOC * HD)  # 256 features
        wq = wqkv_eff[:, 0:D][:, hs]
        wk = wqkv_eff[:, D : 2 * D][:, hs]
        wv = wqkv_eff[:, 2 * D : 3 * D][:, hs]
        wqkv_core = np.concatenate([wq, wk, wv], axis=1).astype(bf)
        wout_core = wout[hs, :].astype(bf)  # [256, D]
        strips = np.stack([
            x[b, sp * 512 + r * P : sp * 512 + (r + 1) * P, :]
            for sp in range(NSP)]).astype(f64) + bvout[None, None, :]
        in_maps.append(dict(
            shared,
            x_b=np.ascontiguousarray(x[b]),
            x_strips=np.ascontiguousarray(
                strips.reshape(4 * P, D).astype(np.float32)),
            wqkv_p=pmajor3(wqkv_core),
            bqk_s=np.ascontiguousarray(np.concatenate(
                [bqkv_eff[0:D][hs], bqkv_eff[D : 2 * D][hs]])
                .astype(np.float32)),
            wout_p=pmajor3(wout_core),
        ))
    return in_maps


def run(inputs, trace=False):
    from concourse.bass_utils import run_bass_kernel_spmd

    attn_mask = np.asarray(inputs["attn_mask"])
    cls, uniq, umap = _mask_structure(attn_mask)
    key = (tuple(sorted(cls.items())), tuple(sorted(umap.items())), len(uniq))
    if key not in _cache:
        _cache[key] = _build(key)
    nc = _cache[key]
    in_maps = _prepare_inputs(inputs)
    kw = {}
    if trace:
        kw = dict(trace=True, trace_cores=list(range(NCORES)))
    res = run_bass_kernel_spmd(nc, in_maps, core_ids=list(range(NCORES)), **kw)
    out = np.empty((B, T, D), np.float32)
    for c in range(NCORES):
        b, r = divmod(c, GROUP)
        strips = res.results[c]["out_strips"].reshape(NSP, P, D)
        for sp in range(NSP):
            out[b, sp * 512 + r * P : sp * 512 + (r + 1) * P, :] = strips[sp]
    return out, res


def kernel(**inputs):
    out, _ = run(inputs, trace=False)
    return out
